# revision 27
# baseline (speedup 1.0000x reference)
"""Trainium2 Bass kernel for nn_ClassicalQuantumAttention.

Data-parallel over batch: 128 batch elems -> 16 per NeuronCore x 8 cores.

v2: batched quantum stage. All 16 batch elems' statevectors live in ONE
tile st_all [128 part = chunks, 2048 free] with free addr
  ri*1024 + b*64 + amp     (ri = re/im, b in 0..15, amp 6 bits, wire w <-> bit 5-w)
Gates become ~8 large tensor_tensor ops (vs ~80 small per-b ops): per-(chunk,b)
cos/sin factors broadcast via stride-0 APs from compact tiles co_all/si_all
[128, 60*16] (layout j*16+b). crx gates with ctrl bit < 5 need small replicated
cos/sin tiles (expanded on gpsimd).

Tail: the shared-param qff ansatz + expvals + normalization collapse into 18
host-precomputed real quadratic forms K_o = [[A,-B],[B,A]] (M_o = U^H O_o U):
qfeat_o(b) = z^T K_o z / (z^T z), computed with PE matmuls + STT-accum reductions.

Classical path in bf16 (host-converted), quantum state in fp16.
"""

import numpy as np
import ml_dtypes
import sys

for _p in ("/opt/trn_rl_repo",):
    if _p not in sys.path:
        sys.path.insert(0, _p)

import concourse.bass as bass
import concourse.tile as tile
from concourse import mybir
from concourse.bass_utils import run_bass_kernel_spmd

F32 = mybir.dt.float32
BF16 = mybir.dt.bfloat16
FP16 = mybir.dt.float16
ALU = mybir.AluOpType
AF = mybir.ActivationFunctionType
AX = mybir.AxisListType

N_CORES = 8
B_TOT = 128
BPC = B_TOT // N_CORES  # 16 batch elems per core
C_IN = 64
T = 2048
D = 256
CH = 16
NC = T // CH  # 128 chunks
NQ = 6
DIM = 64  # 2**6 amplitudes
NB = 16  # batch elems per core (= BPC)
HALF = NB * DIM  # 1024 floats per ri half


# ---------------------------------------------------------------- gate list
def ansatz_gates(n_layers):
    """[(kind, wire-or-(ctrl,tgt), param_idx)] matching reference _ansatz."""
    gates = []
    idx = 0
    for _ in range(n_layers):
        for i in range(NQ):
            gates.append(("rx", i, idx))
            gates.append(("ry", i, idx + 1))
            gates.append(("rz", i, idx + 2))
            idx += 3
        for i in range(NQ):
            gates.append(("crx", (i, (i + 1) % NQ), idx))
            idx += 1
        for i in range(NQ - 1, -1, -1):
            gates.append(("crx", (i, (i - 1) % NQ), idx))
            idx += 1
    return gates


# ------------------------------------------------------------- AP helpers
def state_dims(fixed):
    """(amp_offset, dims) over (b x free amp bits) for one ri half.

    fixed: {bitpos: 0/1}. The b dim (stride 64, count 16) merges with the
    top amp run when that run spans 64; all ansatz cases yield <=2 free dims.
    """
    runs = []
    run = None
    offset = 0
    for p in range(5, -1, -1):
        if p in fixed:
            if run is not None:
                runs.append(run)
                run = None
            offset += fixed[p] << p
        else:
            if run is None:
                run = [1 << p, 2]
            else:
                run = [1 << p, run[1] * 2]
    if run is not None:
        runs.append(run)
    if runs and runs[0][0] * runs[0][1] == 64:
        dims = [[runs[0][0], NB * runs[0][1]]] + runs[1:]
    else:
        dims = [[64, NB]] + runs
    assert len(dims) <= 2, (fixed, dims)
    return offset, dims


def mkap(t, off, dims):
    return bass.AP(tensor=t.tensor, offset=t.offset + off, ap=[list(t.ap[0])] + dims)


def st_ap(t, ri, fixed):
    off, dims = state_dims(fixed)
    return mkap(t, ri * HALF + off, dims)


def cs_ap(t, j, dims):
    """Compact co/si broadcast view matching a merged state view's dims."""
    if len(dims) == 1:
        r0 = dims[0][1] // NB
        return mkap(t, j * NB, [[1, NB], [0, r0]])
    (s0, c0), (s1, c1) = dims
    assert c0 // NB == 1, "needs rep tile"
    return mkap(t, j * NB, [[1, NB], [0, c1]])


def rep_ap(t, dims):
    """Rep-tile (layout b*R+u) view matching state dims."""
    if len(dims) == 1:
        return mkap(t, 0, [[1, dims[0][1]]])
    (s0, c0), (s1, c1) = dims
    return mkap(t, 0, [[1, c0], [0, c1]])


# ------------------------------------------------------------ gate emitters
# Dense gates materialize per-gate cos/sin into contiguous fp16 tiles via one
# ACT copy each (stride-0 source reads); every DVE op then has a packed
# unit-stride last dim -> 2x DVE mode. Sparse layer-1 gates (wires 0-4) keep
# the compact broadcast form (regions too small/scattered to benefit).


def emit_rot_sp(eng, st, B, co, si, kind, p, j, sup):
    """Sparse layer-1 rotation (support-restricted, compact broadcast cs)."""
    fx = {q: 0 for q in sup}
    hoff, hdims = state_dims(fx)
    for ri in (0, 1):
        v = mkap(st, ri * HALF + hoff, hdims)
        eng.tensor_tensor(mkap(B, ri * HALF + hoff, hdims), v, cs_ap(si, j, hdims),
                          ALU.mult)
    for ri in (0, 1):
        v = mkap(st, ri * HALF + hoff, hdims)
        eng.tensor_tensor(v, v, cs_ap(co, j, hdims), ALU.mult)
    if kind == "rx":
        cross = [(0, 0, (1, 1), ALU.add), (1, 0, (0, 1), ALU.subtract),
                 (0, 1, (1, 0), ALU.add), (1, 1, (0, 0), ALU.subtract)]
    elif kind == "ry":
        cross = [(0, 0, (0, 1), ALU.subtract), (1, 0, (1, 1), ALU.subtract),
                 (0, 1, (0, 0), ALU.add), (1, 1, (1, 0), ALU.add)]
    else:  # rz
        cross = [(0, 0, (1, 0), ALU.add), (1, 0, (0, 0), ALU.subtract),
                 (0, 1, (1, 1), ALU.subtract), (1, 1, (0, 1), ALU.add)]
    for ri, k, (bri, bk), op in cross:
        o = st_ap(st, ri, {**fx, p: k})
        eng.tensor_tensor(o, o, st_ap(B, bri, {**fx, p: bk}), op)


def emit_rx_first(eng, st, B, co, si, p, j, sup):
    """RX on bit p when all bits <= p are zero (sparse layer-1 start)."""
    fx = {q: 0 for q in sup}
    off0, d0 = state_dims({**fx, p: 0})
    for ri in (0, 1):
        v = mkap(st, ri * HALF + off0, d0)
        eng.tensor_tensor(mkap(B, ri * HALF + off0, d0), v, cs_ap(si, j, d0),
                          ALU.mult)
    for ri in (0, 1):
        v = mkap(st, ri * HALF + off0, d0)
        eng.tensor_tensor(v, v, cs_ap(co, j, d0), ALU.mult)
    off1, d1 = state_dims({**fx, p: 1})
    eng.tensor_copy(mkap(st, 0 * HALF + off1, d1), mkap(B, 1 * HALF + off0, d0))
    eng.tensor_scalar_mul(mkap(st, 1 * HALF + off1, d1),
                          mkap(B, 0 * HALF + off0, d0), -1.0)


def emit_rot_dense(eng, xeng, st, B, co, si, epool, kind, p, j):
    """Dense rotation: materialized cos/sin, merged both-ri B and *c ops."""
    cfull = epool.tile([128, HALF], FP16, tag="exp", name="cfull")
    sfull = epool.tile([128, HALF], FP16, tag="exp", name="sfull")
    xeng.copy(mkap(cfull, 0, [[1, HALF]]), mkap(co, j * NB, [[1, NB], [0, DIM]]))
    if p == 0 and kind in ("ry", "rz"):
        # sigma-signed sin: s[b] * (-1)^(amp&1)
        xeng.copy(mkap(sfull, 0, [[2, HALF // 2]]),
                  mkap(si, j * NB, [[1, NB], [0, 32]]))
        xeng.mul(mkap(sfull, 1, [[2, HALF // 2]]),
                 mkap(si, j * NB, [[1, NB], [0, 32]]), -1.0)
    else:
        xeng.copy(mkap(sfull, 0, [[1, HALF]]), mkap(si, j * NB, [[1, NB], [0, DIM]]))
    both = lambda t: mkap(t, 0, [[1, 2 * HALF]])
    bc2 = lambda t: mkap(t, 0, [[0, 2], [1, HALF]])
    eng.tensor_tensor(both(B), both(st), bc2(sfull), ALU.mult)
    eng.tensor_tensor(both(st), both(st), bc2(cfull), ALU.mult)
    if p == 0:
        h = lambda t, ri: mkap(t, ri * HALF, [[1, HALF]])
        sw = lambda t, ri: mkap(t, ri * HALF + 1, [[2, HALF // 2], [-1, 2]])
        if kind == "rz":
            eng.tensor_tensor(h(st, 0), h(st, 0), h(B, 1), ALU.add)
            eng.tensor_tensor(h(st, 1), h(st, 1), h(B, 0), ALU.subtract)
        elif kind == "rx":
            eng.tensor_tensor(h(st, 0), h(st, 0), sw(B, 1), ALU.add)
            eng.tensor_tensor(h(st, 1), h(st, 1), sw(B, 0), ALU.subtract)
        else:  # ry
            eng.tensor_tensor(h(st, 0), h(st, 0), sw(B, 0), ALU.add)
            eng.tensor_tensor(h(st, 1), h(st, 1), sw(B, 1), ALU.add)
        return
    if kind == "rx":
        cross = [(0, 0, (1, 1), ALU.add), (1, 0, (0, 1), ALU.subtract),
                 (0, 1, (1, 0), ALU.add), (1, 1, (0, 0), ALU.subtract)]
    elif kind == "ry":
        cross = [(0, 0, (0, 1), ALU.subtract), (1, 0, (1, 1), ALU.subtract),
                 (0, 1, (0, 0), ALU.add), (1, 1, (1, 0), ALU.add)]
    else:  # rz
        cross = [(0, 0, (1, 0), ALU.add), (1, 0, (0, 0), ALU.subtract),
                 (0, 1, (1, 1), ALU.subtract), (1, 1, (0, 1), ALU.add)]
    for ri, k, (bri, bk), op in cross:
        o = st_ap(st, ri, {p: k})
        eng.tensor_tensor(o, o, st_ap(B, bri, {p: bk}), op)


def emit_crx(eng, xeng, nc_gp, st, B, co, si, epool, pc, pt, j):
    coff, cdims = state_dims({pc: 1})
    crep = epool.tile([128, 512], FP16, tag="exp5", name="crep")
    srep = epool.tile([128, 512], FP16, tag="exp5", name="srep")
    # value cos[b] at flat position b*32 + q  (q < 32 = 2^(5-pc) * 2^pc)
    xeng.copy(mkap(crep, 0, [[1, 512]]), mkap(co, j * NB, [[1, NB], [0, 32]]))
    xeng.copy(mkap(srep, 0, [[1, 512]]), mkap(si, j * NB, [[1, NB], [0, 32]]))
    for ri in (0, 1):
        v = mkap(st, ri * HALF + coff, cdims)
        eng.tensor_tensor(mkap(B, ri * HALF + coff, cdims), v,
                          mkap(srep, 0, [[1, 512]]), ALU.mult)
    for ri in (0, 1):
        v = mkap(st, ri * HALF + coff, cdims)
        eng.tensor_tensor(v, v, mkap(crep, 0, [[1, 512]]), ALU.mult)
    if pt == 0 and pc == 1:
        od = [[4, 256], [1, 2]]
        for ro, bi, op in ((0, 1, ALU.add), (1, 0, ALU.subtract)):
            eng.tensor_tensor(
                mkap(st, ro * HALF + coff, od), mkap(st, ro * HALF + coff, od),
                mkap(B, bi * HALF + coff + 1, [[4, 256], [-1, 2]]), op)
        return
    cross = [(0, 0, (1, 1), ALU.add), (1, 0, (0, 1), ALU.subtract),
             (0, 1, (1, 0), ALU.add), (1, 1, (0, 0), ALU.subtract)]
    for ri, k, (bri, bk), op in cross:
        o = st_ap(st, ri, {pc: 1, pt: k})
        eng.tensor_tensor(o, o, st_ap(B, bri, {pc: 1, pt: bk}), op)


def emit_ansatz(eng, xeng, nc_gp, st, B, co, si, epool, n_layers, sparse):
    for gi, (kind, loc, j) in enumerate(ansatz_gates(n_layers)):
        if kind == "crx":
            wc, wt = loc
            emit_crx(eng, xeng, nc_gp, st, B, co, si, epool, 5 - wc, 5 - wt, j)
        else:
            p = 5 - loc
            in_l0 = sparse and gi < 3 * NQ
            if in_l0 and p >= 1:
                sup = set(range(p))
                if kind == "rx":
                    emit_rx_first(eng, st, B, co, si, p, j, sup)
                else:
                    emit_rot_sp(eng, st, B, co, si, kind, p, j, sup)
            else:
                emit_rot_dense(eng, xeng, st, B, co, si, epool, kind, p, j)


def _split_multi_waits(nc):
    """This walrus build allows at most ONE sync-wait per instruction."""
    ctr = [0]
    for f in nc.m.functions:
        for b in f.blocks:
            new = []
            for inst in b.instructions:
                si = inst.sync_info
                if si is not None and len(si.on_wait) > 1:
                    waits = list(si.on_wait)
                    for w in waits[:-1]:
                        ctr[0] += 1
                        nop = mybir.InstNoOp(
                            name=f"wsplit-{ctr[0]}",
                            ins=[],
                            outs=[],
                            engine=inst.engine,
                            sync_info=mybir.SyncInfo(on_wait=[w], on_update=[]),
                        )
                        new.append(nop)
                    inst.sync_info = mybir.SyncInfo(
                        on_wait=[waits[-1]], on_update=list(si.on_update)
                    )
                new.append(inst)
            b.instructions = new


# ---------------------------------------------------------------- program
def build_program(split_waits=True):
    nc = bass.Bass()

    for v in (float(np.pi / 2), 1e-5, 0.25, float(0.25 + np.pi / 2)):
        t = nc.alloc_sbuf_tensor(f"const-f32-{v}", [128, 1], F32)
        nc.gpsimd.memset(t.ap(), v)
        nc.const_aps.aps[(F32, v)] = t.ap()
    nc.all_engine_barrier()

    # ---- dram I/O (per core) ----
    xs = nc.declare_dram_parameter("xs", [BPC, C_IN, T], BF16, isOutput=False)
    xp = nc.declare_dram_parameter("xp", [BPC, NC, CH * C_IN], FP16, isOutput=False)
    wfb = nc.declare_dram_parameter("wfb", [C_IN + 1, 128], BF16, isOutput=False)
    aw2 = nc.declare_dram_parameter("aw2", [128, 1], BF16, isOutput=False)
    ewb = nc.declare_dram_parameter("ewb", [C_IN + 1, D], BF16, isOutput=False)
    pjw = nc.declare_dram_parameter("pjw", [128, 120], BF16, isOutput=False)
    pjb = nc.declare_dram_parameter("pjb", [1, 60], BF16, isOutput=False)
    cf3 = nc.declare_dram_parameter("cf3", [NC, 3], FP16, isOutput=False)
    kbig = nc.declare_dram_parameter("kbig", [128, 18 * 128], FP16, isOutput=False)
    owb = nc.declare_dram_parameter("owb", [19, D], F32, isOutput=False)
    lng = nc.declare_dram_parameter("lng", [BPC, D], F32, isOutput=False)
    lnb = nc.declare_dram_parameter("lnb", [BPC, D], F32, isOutput=False)
    cw1 = nc.declare_dram_parameter("cw1", [128, 2 * D], F32, isOutput=False)
    cb1 = nc.declare_dram_parameter("cb1", [1, D], F32, isOutput=False)
    cw2 = nc.declare_dram_parameter("cw2", [128, 4], F32, isOutput=False)
    cb2 = nc.declare_dram_parameter("cb2", [1, 2], F32, isOutput=False)
    idn = nc.declare_dram_parameter("idn", [128, 128], F32, isOutput=False)
    idn16 = nc.declare_dram_parameter("idn16", [128, 128], FP16, isOutput=False)
    out = nc.declare_dram_parameter("out", [BPC, 2], F32, isOutput=True)

    with tile.TileContext(nc) as tc:
        with (
            tc.tile_pool(name="const", bufs=1) as cp,
            tc.tile_pool(name="xbuf", bufs=3) as xpool,
            tc.tile_pool(name="xpbuf", bufs=3) as xppool,
            tc.tile_pool(name="tanh", bufs=3) as thpool,
            tc.tile_pool(name="exp", bufs=12) as epool,
            tc.tile_pool(name="small", bufs=4) as sm,
            tc.tile_pool(name="ps_h", bufs=3, space="PSUM") as ps_h,
            tc.tile_pool(name="ps_s", bufs=1, space="PSUM") as ps_s,
            tc.tile_pool(name="ps_m", bufs=2, space="PSUM") as ps_m,
            tc.tile_pool(name="ps_t", bufs=2, space="PSUM") as ps_t,
        ):
            # ---------------- constants into SBUF ----------------
            def cload(name, dram, shape, dt=F32):
                t = cp.tile(shape, dt, tag=name, name=name)
                nc.sync.dma_start(out=t, in_=dram[:, :])
                return t

            wfb_s = cload("wfb", wfb, [C_IN + 1, 128], BF16)
            aw2_s = cload("aw2", aw2, [128, 1], BF16)

            # per-b input buffers + prefetch of the first batch elems before
            # the remaining (later-needed) constant loads hit the DMA queue
            x_sb = [xpool.tile([C_IN + 1, T], BF16, tag="x", name=f"xsb{i}") for i in range(3)]
            xp_sb = [xppool.tile([NC, CH * C_IN], FP16, tag="xp", name=f"xpsb{i}") for i in range(3)]
            xwt_sb = [xppool.tile([C_IN + 1, NC], BF16, tag="xwt", name=f"xwtsb{i}") for i in range(3)]
            for i in range(3):
                nc.vector.memset(x_sb[i][C_IN : C_IN + 1, :], 1.0)
                nc.vector.memset(xwt_sb[i][C_IN : C_IN + 1, :], 1.0)
            for b0 in range(2):
                nc.sync.dma_start(out=x_sb[b0][0:C_IN, :], in_=xs[b0, :, :])
                nc.sync.dma_start(out=xp_sb[b0], in_=xp[b0, :, :])

            idn16_s = cload("idn16", idn16, [128, 128], FP16)
            ewb_s = cload("ewb", ewb, [C_IN + 1, D], BF16)
            pjw_s = cload("pjw", pjw, [128, 120], BF16)
            pjb_s = cload("pjb", pjb, [1, 60], BF16)
            cf3_s = cload("cf3", cf3, [NC, 3], FP16)

            ones = cp.tile([1, 128], F32, tag="ones")
            nc.vector.memset(ones, 1.0)
            ones_bf = cp.tile([1, 128], BF16, tag="ones_bf")
            nc.vector.memset(ones_bf, 1.0)

            # per-b score tiles (double-buffered via pool tags)
            scb_t = [sm.tile([NC, CH], F32, tag="scb", name=f"scb{i}") for i in range(3)]
            wb_t = [sm.tile([NC, CH], FP16, tag="wb", name=f"wb{i}") for i in range(3)]

            # batched quantum state + scratch + compact trig tiles
            st_all = cp.tile([128, 2 * HALF], FP16, tag="stall")
            B_all = cp.tile([128, 2 * HALF], FP16, tag="Ball")
            co_all = cp.tile([128, 60 * NB], FP16, tag="coall")
            si_all = cp.tile([128, 60 * NB], FP16, tag="siall")

            # init state |0..0> for all (chunk, b)
            nc.vector.memset(st_all, 0.0)
            nc.vector.memset(mkap(st_all, 0, [[64, NB], [1, 1]]), 1.0)

            qfeat = cp.tile([BPC, 19], F32, tag="qfeat")
            nc.vector.memset(qfeat[:, 18:19], 1.0)
            zrow = cp.tile([1, 2 * HALF], F32, tag="zrow")
            Zt = cp.tile([128, NB], FP16, tag="Zt")
            Zb = cp.tile([BPC, 128], F32, tag="Zb")

            # ================= classical per-b =================
            for b in range(BPC):
                g, bb = b // 8, b % 8
                if bb == 0:
                    par_g = ps_t.tile([NC, 8 * 60], F32, tag="t")
                xb = x_sb[b % 3]
                xpb = xp_sb[b % 3]
                if b >= 2:
                    nc.sync.dma_start(out=xb[0:C_IN, :], in_=xs[b, :, :])
                    nc.sync.dma_start(out=xpb, in_=xp[b, :, :])

                scb = scb_t[b % 3]
                th = thpool.tile([128, T], BF16, tag="th")
                for blk in range(4):
                    hp = ps_h.tile([128, 512], F32, tag="hp")
                    nc.tensor.matmul(
                        hp,
                        wfb_s[:, :],
                        xb[:, blk * 512 : (blk + 1) * 512],
                        start=True,
                        stop=True,
                    )
                    nc.scalar.activation(
                        th[:, blk * 512 : (blk + 1) * 512], hp, AF.Tanh
                    )
                    sc = ps_s.tile([1, 512], F32, tag="sc")
                    nc.tensor.matmul(
                        sc,
                        aw2_s[:, :],
                        th[:, blk * 512 : (blk + 1) * 512],
                        start=True,
                        stop=True,
                    )
                    ssc = sm.tile([1, 512], F32, tag="ssc", name="ssc")
                    if blk % 2 == 0:
                        nc.vector.tensor_scalar_mul(ssc, sc, 1.0)
                    else:
                        nc.scalar.copy(ssc, sc)
                    src_ = ssc.rearrange("p (n k) -> p n k", n=32, k=CH)
                    nc.sync.dma_start(
                        out=scb[blk * 32 : (blk + 1) * 32, :], in_=src_
                    )

                # per-b softmax
                esc = sm.tile([NC, CH], F32, tag="esc", name="esc")
                nc.scalar.activation(esc, scb, AF.Exp)
                ssum = sm.tile([NC, 1], F32, tag="ssum")
                nc.vector.tensor_reduce(ssum, esc, AX.X, ALU.add)
                rsum = sm.tile([NC, 1], F32, tag="rsum")
                nc.vector.reciprocal(rsum, ssum)
                wb = wb_t[b % 3]
                nc.gpsimd.tensor_scalar_mul(wb, esc, rsum[:, 0:1])

                # chunk path
                xwtmp = sm.tile([NC, CH * C_IN], FP16, tag="xwtmp", name="xwtmp")
                wv = mkap(wb, 0, [[0, C_IN], [1, CH]])
                nc.vector.tensor_tensor(
                    mkap(xwtmp, 0, [[1, CH * C_IN]]),
                    mkap(xpb, 0, [[1, CH * C_IN]]),
                    wv,
                    ALU.mult,
                )
                xw = sm.tile([NC, C_IN], FP16, tag="xw")
                with nc.allow_low_precision("16-elem fp16 sums, tol 2e-2"):
                    nc.vector.tensor_reduce(
                        xw,
                        mkap(xwtmp, 0, [[CH, C_IN], [1, CH]]),
                        AX.X,
                        ALU.add,
                    )
                xwt_ps = ps_m.tile([C_IN, NC], FP16, tag="m")
                nc.tensor.transpose(xwt_ps, xw, idn16_s)
                xwt = xwt_sb[b % 3]
                nc.vector.tensor_copy(xwt[0:C_IN, :], xwt_ps)
                cht = [None, None]
                for h in range(2):
                    chp = ps_m.tile([128, NC], F32, tag="m")
                    nc.tensor.matmul(
                        chp,
                        ewb_s[:, h * 128 : (h + 1) * 128],
                        xwt,
                        start=True,
                        stop=True,
                    )
                    cht[h] = sm.tile([128, NC], BF16, tag=f"cht{h}", name=f"cht{h}")
                    if h == 0:
                        nc.vector.tensor_copy(cht[h], chp)
                    else:
                        nc.scalar.copy(cht[h], chp)
                par = par_g[:, bb * 60 : (bb + 1) * 60]
                nc.tensor.matmul(
                    par, cht[0], pjw_s[:, 0:60],
                    start=True, stop=False, skip_group_check=True,
                )
                nc.tensor.matmul(
                    par, cht[1], pjw_s[:, 60:120],
                    start=False, stop=False, skip_group_check=True,
                )
                nc.tensor.matmul(
                    par, ones_bf, pjb_s,
                    start=False, stop=True, skip_group_check=True,
                )

                if bb == 7:
                    # theta = sigmoid(z) = 0.5 + 0.5*tanh(z/2); Tanh+Sin share
                    # an act table set.
                    tg = sm.tile([NC, 8 * 60], FP16, tag="tg", name="tg")
                    nc.scalar.activation(tg, par_g, AF.Tanh, scale=0.5)
                    co_dst = mkap(co_all, g * 8, [[1, 8], [NB, 60]])
                    si_dst = mkap(si_all, g * 8, [[1, 8], [NB, 60]])
                    tg_v = mkap(tg, 0, [[60, 8], [1, 60]])
                    nc.scalar.activation(
                        co_dst, tg_v, AF.Sin,
                        bias=float(0.25 + np.pi / 2), scale=0.25,
                    )
                    nc.scalar.activation(
                        si_dst, tg_v, AF.Sin, bias=0.25, scale=0.25
                    )

            idn_s = cload("idn", idn, [128, 128])
            kbig_s = cload("kbig", kbig, [128, 18 * 128], FP16)
            owb_s = cload("owb", owb, [19, D])
            lng_s = cload("lng", lng, [BPC, D])
            lnb_s = cload("lnb", lnb, [BPC, D])
            cw1_s = cload("cw1", cw1, [128, 2 * D])
            cb1_s = cload("cb1", cb1, [1, D])
            cw2_s = cload("cw2", cw2, [128, 4])
            cb2_s = cload("cb2", cb2, [1, 2])

            # ================= batched quantum stage 1 =================
            emit_ansatz(nc.vector, nc.scalar, nc.gpsimd, st_all, B_all,
                        co_all, si_all, epool, 2, sparse=True)

            # ================= LCU -> z rows =================
            # zx = cr^T st_re + (-ci)^T st_im ; zy = ci^T st_re + cr^T st_im
            for h in range(2):
                zx_ps = ps_m.tile([1, 512], F32, tag="m")
                nc.tensor.matmul(zx_ps, cf3_s[:, 0:1],
                                 st_all[:, h * 512 : (h + 1) * 512],
                                 start=True, stop=False)
                nc.tensor.matmul(zx_ps, cf3_s[:, 2:3],
                                 st_all[:, HALF + h * 512 : HALF + (h + 1) * 512],
                                 start=False, stop=True)
                nc.vector.tensor_scalar_mul(zrow[:, h * 512 : (h + 1) * 512], zx_ps, 1.0)
            for h in range(2):
                zy_ps = ps_m.tile([1, 512], F32, tag="m")
                nc.tensor.matmul(zy_ps, cf3_s[:, 1:2],
                                 st_all[:, h * 512 : (h + 1) * 512],
                                 start=True, stop=False)
                nc.tensor.matmul(zy_ps, cf3_s[:, 0:1],
                                 st_all[:, HALF + h * 512 : HALF + (h + 1) * 512],
                                 start=False, stop=True)
                nc.vector.tensor_scalar_mul(
                    zrow[:, HALF + h * 512 : HALF + (h + 1) * 512], zy_ps, 1.0)

            # zrow [1, (ri,b,amp)] -> Zb [16 part=b, 128=(ri,amp)] via 2 DMAs
            for ri in (0, 1):
                src = bass.AP(
                    tensor=zrow.tensor, offset=zrow.offset + ri * HALF,
                    ap=[list(zrow.ap[0])] + [[DIM, NB], [1, DIM]],
                )
                nc.sync.dma_start(out=Zb[:, ri * DIM : (ri + 1) * DIM], in_=src)
            # Zt [128, 16] = Zb^T
            zt_ps = ps_m.tile([128, BPC], F32, tag="m")
            nc.tensor.transpose(zt_ps, Zb, idn_s[0:BPC, 0:BPC])
            nc.vector.tensor_copy(Zt, zt_ps)

            # n = z.z per b; qfeat_o = (z^T K_o z) / n
            nsum = sm.tile([BPC, 1], F32, tag="nsum")
            scr0 = sm.tile([BPC, 128], F32, tag="scr0", name="scr0")
            nc.vector.scalar_tensor_tensor(
                scr0, Zb, 1.0, Zb, ALU.mult, ALU.mult, accum_out=nsum
            )
            scr = [sm.tile([BPC, 128], F32, tag=f"scrk{i}", name=f"scrk{i}")
                   for i in range(4)]
            for blk in range(5):
                n0 = blk * 4
                nobs = min(4, 18 - n0)
                y_ps = ps_t.tile([BPC, 512], F32, tag="t")
                nc.tensor.matmul(
                    y_ps[:, 0 : nobs * 128],
                    Zt[:, :],
                    kbig_s[:, n0 * 128 : (n0 + nobs) * 128],
                    start=True, stop=True,
                )
                for o in range(nobs):
                    nc.vector.scalar_tensor_tensor(
                        scr[o % 4],
                        y_ps[:, o * 128 : (o + 1) * 128],
                        1.0,
                        Zb,
                        ALU.mult, ALU.mult,
                        accum_out=qfeat[:, n0 + o : n0 + o + 1],
                    )
            rn = sm.tile([BPC, 1], F32, tag="rn")
            nc.vector.reciprocal(rn, nsum)
            nc.vector.tensor_scalar_mul(qfeat[:, 0:18], qfeat[:, 0:18], rn)

            # ================= tail =================
            qfT_ps = ps_m.tile([19, BPC], F32, tag="m")
            nc.tensor.transpose(qfT_ps, qfeat, idn_s[0:BPC, 0:BPC])
            qfT = sm.tile([19, BPC], F32, tag="qfTs")
            nc.vector.tensor_copy(qfT, qfT_ps)
            o1 = ps_t.tile([BPC, D], F32, tag="t")
            nc.tensor.matmul(o1, qfT, owb_s, start=True, stop=True)

            stats = sm.tile([BPC, 6], F32, tag="stats")
            nc.vector.bn_stats(stats, o1)
            mv = sm.tile([BPC, 2], F32, tag="mv")
            nc.vector.bn_aggr(mv, stats)
            sdv = sm.tile([BPC, 1], F32, tag="sdv")
            nc.scalar.activation(sdv, mv[:, 1:2], AF.Sqrt, bias=1e-5)
            rstd = sm.tile([BPC, 1], F32, tag="rstd")
            nc.vector.reciprocal(rstd, sdv)
            ln1 = sm.tile([BPC, D], F32, tag="ln1")
            nc.vector.tensor_scalar(
                ln1, o1, mv[:, 0:1], rstd, ALU.subtract, ALU.mult
            )
            ln2 = sm.tile([BPC, D], F32, tag="ln2")
            nc.vector.tensor_tensor(ln2, ln1, lng_s, ALU.mult)
            nc.vector.tensor_tensor(ln2, ln2, lnb_s, ALU.add)

            # cls layer 1
            lnT = [None, None]
            for h in range(2):
                lnT_ps = ps_m.tile([128, BPC], F32, tag="m")
                nc.tensor.transpose(
                    lnT_ps, ln2[:, h * 128 : (h + 1) * 128], idn_s[0:BPC, 0:BPC]
                )
                lnT[h] = sm.tile([128, BPC], F32, tag=f"lnT{h}", name=f"lnT{h}")
                nc.vector.tensor_copy(lnT[h], lnT_ps)
            h2p = ps_t.tile([BPC, D], F32, tag="t")
            nc.tensor.matmul(h2p, lnT[0], cw1_s[:, 0:D], start=True, stop=False)
            nc.tensor.matmul(
                h2p, lnT[1], cw1_s[:, D : 2 * D], start=False, stop=False
            )
            nc.tensor.matmul(
                h2p, ones[:, 0:BPC], cb1_s, start=False, stop=True
            )
            h2 = sm.tile([BPC, D], F32, tag="h2")
            nc.scalar.activation(h2, h2p, AF.Relu)

            # cls layer 2
            h2T = [None, None]
            for h in range(2):
                h2T_ps = ps_m.tile([128, BPC], F32, tag="m")
                nc.tensor.transpose(
                    h2T_ps, h2[:, h * 128 : (h + 1) * 128], idn_s[0:BPC, 0:BPC]
                )
                h2T[h] = sm.tile([128, BPC], F32, tag=f"h2T{h}", name=f"h2T{h}")
                nc.vector.tensor_copy(h2T[h], h2T_ps)
            lg = ps_t.tile([BPC, 2], F32, tag="t")
            nc.tensor.matmul(lg, h2T[0], cw2_s[:, 0:2], start=True, stop=False)
            nc.tensor.matmul(lg, h2T[1], cw2_s[:, 2:4], start=False, stop=False)
            nc.tensor.matmul(lg, ones[:, 0:BPC], cb2_s, start=False, stop=True)
            lgs = sm.tile([BPC, 2], F32, tag="lgs")
            nc.vector.tensor_copy(lgs, lg)
            nc.sync.dma_start(out=out[:, :], in_=lgs)

    if split_waits:
        _split_multi_waits(nc)
    return nc


_NC_CACHE = {}


def _get_program():
    if "nc" not in _NC_CACHE:
        _NC_CACHE["nc"] = build_program()
    return _NC_CACHE["nc"]


# ------------------------------------------------- host-side qff K matrices
def _np_rx(t):
    c, s = np.cos(t / 2), np.sin(t / 2)
    return np.array([[c, -1j * s], [-1j * s, c]])


def _np_ry(t):
    c, s = np.cos(t / 2), np.sin(t / 2)
    return np.array([[c, -s], [s, c]], complex)


def _np_rz(t):
    e = np.exp(-0.5j * t)
    return np.array([[e, 0], [0, np.conj(e)]])


def _np_apply_1q(state, gate, wire):
    # state [B, 64]; wire 0 = MSB
    dl, dr = 2 ** wire, 2 ** (NQ - 1 - wire)
    s = state.reshape(-1, dl, 2, dr)
    s = np.einsum('kj,bljr->blkr', gate, s)
    return s.reshape(-1, DIM)


def _np_apply_crx(state, theta, control, target):
    s = state.reshape((-1,) + (2,) * NQ)
    s = np.moveaxis(s, (control + 1, target + 1), (1, 2))
    g = _np_rx(theta)
    s1 = np.einsum('kj,bj...->bk...', g, s[:, 1])
    s = np.concatenate([s[:, :1], s1[:, None]], axis=1)
    s = np.moveaxis(s, (1, 2), (control + 1, target + 1))
    return s.reshape(-1, DIM)


def _np_ansatz(state, params, n_layers):
    idx = 0
    for _ in range(n_layers):
        for i in range(NQ):
            state = _np_apply_1q(state, _np_rx(params[idx]), i)
            state = _np_apply_1q(state, _np_ry(params[idx + 1]), i)
            state = _np_apply_1q(state, _np_rz(params[idx + 2]), i)
            idx += 3
        for i in range(NQ):
            state = _np_apply_crx(state, params[idx], i, (i + 1) % NQ)
            idx += 1
        for i in range(NQ - 1, -1, -1):
            state = _np_apply_crx(state, params[idx], i, (i - 1) % NQ)
            idx += 1
    return state


def _qff_k_matrices(qff_params):
    """Kbig [128, 18*128]: real quadratic forms for U^H O_o U, o = X0..X5,Y0..Y5,Z0..Z5."""
    U = _np_ansatz(np.eye(DIM, dtype=complex), np.asarray(qff_params, np.float64), 1)
    U = U.T  # rows of _np_ansatz output are evolved basis states -> columns of U
    PX = np.array([[0, 1], [1, 0]], complex)
    PY = np.array([[0, -1j], [1j, 0]])
    PZ = np.array([[1, 0], [0, -1]], complex)
    I2 = np.eye(2)
    blocks = []
    for P in (PX, PY, PZ):
        for i in range(NQ):
            O = np.array([[1.0 + 0j]])
            for w in range(NQ):
                O = np.kron(O, P if w == i else I2)
            M = U.conj().T @ O @ U
            A, Bm = M.real, M.imag
            K = np.block([[A, -Bm], [Bm, A]])
            blocks.append(K)
    return np.concatenate(blocks, axis=1).astype(np.float32)


def host_prep(inputs):
    """Host-side parameter folding -> per-core input maps."""
    f32 = np.float32
    x = np.asarray(inputs["x"], f32)
    emb_w = np.asarray(inputs["emb_w"], np.float64)
    emb_b = np.asarray(inputs["emb_b"], np.float64)
    att_w1 = np.asarray(inputs["att_w1"], np.float64)
    att_b1 = np.asarray(inputs["att_b1"], np.float64)

    bf16 = ml_dtypes.bfloat16
    wfold = (emb_w @ att_w1).astype(f32)  # [64, 128]
    bfold = (emb_b @ att_w1 + att_b1).astype(f32)  # [128]
    wfb = np.concatenate([wfold, bfold[None, :]], 0).astype(bf16)  # [65, 128]

    ewb = np.concatenate(
        [emb_w.astype(f32), emb_b.astype(f32)[None, :]], 0
    ).astype(bf16)  # [65, 256]

    pw = np.asarray(inputs["proj_w"], f32)  # [256, 60]
    pjw = np.concatenate([pw[0:128, :], pw[128:256, :]], 1).astype(bf16)  # [128, 120]

    cr = np.asarray(inputs["mix_re"], np.float64)
    ci = np.asarray(inputs["mix_im"], np.float64)
    den = np.sqrt(cr * cr + ci * ci).sum() + 1e-8
    cf3 = np.stack([cr / den, ci / den, -ci / den], 1).astype(np.float16)  # [128, 3]

    kbig = _qff_k_matrices(inputs["qff_params"]).astype(np.float16)  # [128, 2304]

    owb = np.concatenate(
        [np.asarray(inputs["out_w"], f32), np.asarray(inputs["out_b"], f32)[None, :]],
        0,
    )  # [19, 256]
    lng = np.broadcast_to(np.asarray(inputs["ln_g"], f32), (BPC, D)).copy()
    lnb = np.broadcast_to(np.asarray(inputs["ln_b"], f32), (BPC, D)).copy()
    w1 = np.asarray(inputs["cls_w1"], f32)
    cw1 = np.concatenate([w1[0:128, :], w1[128:256, :]], 1)  # [128, 512]
    cb1 = np.asarray(inputs["cls_b1"], f32)[None, :]
    w2 = np.asarray(inputs["cls_w2"], f32)
    cw2 = np.concatenate([w2[0:128, :], w2[128:256, :]], 1)  # [128, 4]
    cb2 = np.asarray(inputs["cls_b2"], f32)[None, :]
    idn = np.eye(128, dtype=f32)
    idn16 = np.eye(128, dtype=np.float16)
    pjb = np.asarray(inputs["proj_b"], f32)[None, :]

    shared = dict(
        wfb=wfb, aw2=np.asarray(inputs["att_w2"], f32).astype(bf16), ewb=ewb,
        pjw=pjw, pjb=pjb.astype(bf16), cf3=cf3, kbig=kbig, owb=owb, lng=lng,
        lnb=lnb, cw1=cw1, cb1=cb1, cw2=cw2, cb2=cb2, idn=idn, idn16=idn16,
    )

    in_maps = []
    for c in range(N_CORES):
        xc = x[c * BPC : (c + 1) * BPC]  # [16, 64, 2048]
        # xperm[b, nc, c*16+k] = x[b, c, nc*16+k]
        xp_c = np.ascontiguousarray(
            xc.reshape(BPC, C_IN, NC, CH).transpose(0, 2, 1, 3).reshape(
                BPC, NC, CH * C_IN
            )
        )
        m = dict(shared)
        m["xs"] = np.ascontiguousarray(xc).astype(bf16)
        m["xp"] = xp_c.astype(np.float16)
        in_maps.append(m)
    return in_maps


def kernel(**inputs):
    nc = _get_program()
    in_maps = host_prep(inputs)
    res = run_bass_kernel_spmd(nc, in_maps, core_ids=list(range(N_CORES)))
    outs = [res.results[c]["out"] for c in range(N_CORES)]
    return np.concatenate(outs, 0).astype(np.float32)


if __name__ == "__main__":
    nc = build_program()
    print("program built ok")


# revision 28
# speedup vs baseline: 1.0490x; 1.0490x over previous
"""Trainium2 Bass kernel for nn_ClassicalQuantumAttention.

Data-parallel over batch: 128 batch elems -> 16 per NeuronCore x 8 cores.

v2: batched quantum stage. All 16 batch elems' statevectors live in ONE
tile st_all [128 part = chunks, 2048 free] with free addr
  ri*1024 + b*64 + amp     (ri = re/im, b in 0..15, amp 6 bits, wire w <-> bit 5-w)
Gates become ~8 large tensor_tensor ops (vs ~80 small per-b ops): per-(chunk,b)
cos/sin factors broadcast via stride-0 APs from compact tiles co_all/si_all
[128, 60*16] (layout j*16+b). crx gates with ctrl bit < 5 need small replicated
cos/sin tiles (expanded on gpsimd).

Tail: the shared-param qff ansatz + expvals + normalization collapse into 18
host-precomputed real quadratic forms K_o = [[A,-B],[B,A]] (M_o = U^H O_o U):
qfeat_o(b) = z^T K_o z / (z^T z), computed with PE matmuls + STT-accum reductions.

Classical path in bf16 (host-converted), quantum state in fp16.
"""

import numpy as np
import ml_dtypes
import sys

for _p in ("/opt/trn_rl_repo",):
    if _p not in sys.path:
        sys.path.insert(0, _p)

import concourse.bass as bass
import concourse.tile as tile
from concourse import mybir
from concourse.bass_utils import run_bass_kernel_spmd

F32 = mybir.dt.float32
BF16 = mybir.dt.bfloat16
FP16 = mybir.dt.float16
ALU = mybir.AluOpType
AF = mybir.ActivationFunctionType
AX = mybir.AxisListType

N_CORES = 8
B_TOT = 128
BPC = B_TOT // N_CORES  # 16 batch elems per core
C_IN = 64
T = 2048
D = 256
CH = 16
NC = T // CH  # 128 chunks
NQ = 6
DIM = 64  # 2**6 amplitudes
NB = 16  # batch elems per core (= BPC)
HALF = NB * DIM  # 1024 floats per ri half


# ---------------------------------------------------------------- gate list
def ansatz_gates(n_layers):
    """[(kind, wire-or-(ctrl,tgt), param_idx)] matching reference _ansatz."""
    gates = []
    idx = 0
    for _ in range(n_layers):
        for i in range(NQ):
            gates.append(("rx", i, idx))
            gates.append(("ry", i, idx + 1))
            gates.append(("rz", i, idx + 2))
            idx += 3
        for i in range(NQ):
            gates.append(("crx", (i, (i + 1) % NQ), idx))
            idx += 1
        for i in range(NQ - 1, -1, -1):
            gates.append(("crx", (i, (i - 1) % NQ), idx))
            idx += 1
    return gates


# ------------------------------------------------------------- AP helpers
def state_dims(fixed):
    """(amp_offset, dims) over (b x free amp bits) for one ri half.

    fixed: {bitpos: 0/1}. The b dim (stride 64, count 16) merges with the
    top amp run when that run spans 64; all ansatz cases yield <=2 free dims.
    """
    runs = []
    run = None
    offset = 0
    for p in range(5, -1, -1):
        if p in fixed:
            if run is not None:
                runs.append(run)
                run = None
            offset += fixed[p] << p
        else:
            if run is None:
                run = [1 << p, 2]
            else:
                run = [1 << p, run[1] * 2]
    if run is not None:
        runs.append(run)
    if runs and runs[0][0] * runs[0][1] == 64:
        dims = [[runs[0][0], NB * runs[0][1]]] + runs[1:]
    else:
        dims = [[64, NB]] + runs
    assert len(dims) <= 2, (fixed, dims)
    return offset, dims


def mkap(t, off, dims):
    return bass.AP(tensor=t.tensor, offset=t.offset + off, ap=[list(t.ap[0])] + dims)


def st_ap(t, ri, fixed):
    off, dims = state_dims(fixed)
    return mkap(t, ri * HALF + off, dims)


def cs_ap(t, j, dims):
    """Compact co/si broadcast view matching a merged state view's dims."""
    if len(dims) == 1:
        r0 = dims[0][1] // NB
        return mkap(t, j * NB, [[1, NB], [0, r0]])
    (s0, c0), (s1, c1) = dims
    assert c0 // NB == 1, "needs rep tile"
    return mkap(t, j * NB, [[1, NB], [0, c1]])


def rep_ap(t, dims):
    """Rep-tile (layout b*R+u) view matching state dims."""
    if len(dims) == 1:
        return mkap(t, 0, [[1, dims[0][1]]])
    (s0, c0), (s1, c1) = dims
    return mkap(t, 0, [[1, c0], [0, c1]])


# ------------------------------------------------------------ gate emitters
# Dense gates materialize per-gate cos/sin into contiguous fp16 tiles via one
# ACT copy each (stride-0 source reads); every DVE op then has a packed
# unit-stride last dim -> 2x DVE mode. Sparse layer-1 gates (wires 0-4) keep
# the compact broadcast form (regions too small/scattered to benefit).


def emit_rot_sp(eng, st, B, co, si, kind, p, j, sup):
    """Sparse layer-1 rotation (support-restricted, compact broadcast cs)."""
    fx = {q: 0 for q in sup}
    hoff, hdims = state_dims(fx)
    for ri in (0, 1):
        v = mkap(st, ri * HALF + hoff, hdims)
        eng.tensor_tensor(mkap(B, ri * HALF + hoff, hdims), v, cs_ap(si, j, hdims),
                          ALU.mult)
    for ri in (0, 1):
        v = mkap(st, ri * HALF + hoff, hdims)
        eng.tensor_tensor(v, v, cs_ap(co, j, hdims), ALU.mult)
    if kind == "rx":
        cross = [(0, 0, (1, 1), ALU.add), (1, 0, (0, 1), ALU.subtract),
                 (0, 1, (1, 0), ALU.add), (1, 1, (0, 0), ALU.subtract)]
    elif kind == "ry":
        cross = [(0, 0, (0, 1), ALU.subtract), (1, 0, (1, 1), ALU.subtract),
                 (0, 1, (0, 0), ALU.add), (1, 1, (1, 0), ALU.add)]
    else:  # rz
        cross = [(0, 0, (1, 0), ALU.add), (1, 0, (0, 0), ALU.subtract),
                 (0, 1, (1, 1), ALU.subtract), (1, 1, (0, 1), ALU.add)]
    for ri, k, (bri, bk), op in cross:
        o = st_ap(st, ri, {**fx, p: k})
        eng.tensor_tensor(o, o, st_ap(B, bri, {**fx, p: bk}), op)


def emit_rx_first(eng, st, B, co, si, p, j, sup):
    """RX on bit p when all bits <= p are zero (sparse layer-1 start)."""
    fx = {q: 0 for q in sup}
    off0, d0 = state_dims({**fx, p: 0})
    for ri in (0, 1):
        v = mkap(st, ri * HALF + off0, d0)
        eng.tensor_tensor(mkap(B, ri * HALF + off0, d0), v, cs_ap(si, j, d0),
                          ALU.mult)
    for ri in (0, 1):
        v = mkap(st, ri * HALF + off0, d0)
        eng.tensor_tensor(v, v, cs_ap(co, j, d0), ALU.mult)
    off1, d1 = state_dims({**fx, p: 1})
    eng.tensor_copy(mkap(st, 0 * HALF + off1, d1), mkap(B, 1 * HALF + off0, d0))
    eng.tensor_scalar_mul(mkap(st, 1 * HALF + off1, d1),
                          mkap(B, 0 * HALF + off0, d0), -1.0)


def emit_rot_dense(eng, xeng, st, B, co, si, epool, kind, p, j):
    """Dense rotation: materialized cos/sin, merged both-ri B and *c ops."""
    cfull = epool.tile([128, HALF], FP16, tag="exp", name="cfull")
    sfull = epool.tile([128, HALF], FP16, tag="exp", name="sfull")
    xeng.copy(mkap(cfull, 0, [[1, HALF]]), mkap(co, j * NB, [[1, NB], [0, DIM]]))
    if p == 0 and kind in ("ry", "rz"):
        # sigma-signed sin: s[b] * (-1)^(amp&1)
        xeng.copy(mkap(sfull, 0, [[2, HALF // 2]]),
                  mkap(si, j * NB, [[1, NB], [0, 32]]))
        xeng.mul(mkap(sfull, 1, [[2, HALF // 2]]),
                 mkap(si, j * NB, [[1, NB], [0, 32]]), -1.0)
    else:
        xeng.copy(mkap(sfull, 0, [[1, HALF]]), mkap(si, j * NB, [[1, NB], [0, DIM]]))
    both = lambda t: mkap(t, 0, [[1, 2 * HALF]])
    bc2 = lambda t: mkap(t, 0, [[0, 2], [1, HALF]])
    eng.tensor_tensor(both(B), both(st), bc2(sfull), ALU.mult)
    eng.tensor_tensor(both(st), both(st), bc2(cfull), ALU.mult)
    if p == 0:
        h = lambda t, ri: mkap(t, ri * HALF, [[1, HALF]])
        sw = lambda t, ri: mkap(t, ri * HALF + 1, [[2, HALF // 2], [-1, 2]])
        if kind == "rz":
            eng.tensor_tensor(h(st, 0), h(st, 0), h(B, 1), ALU.add)
            eng.tensor_tensor(h(st, 1), h(st, 1), h(B, 0), ALU.subtract)
        elif kind == "rx":
            eng.tensor_tensor(h(st, 0), h(st, 0), sw(B, 1), ALU.add)
            eng.tensor_tensor(h(st, 1), h(st, 1), sw(B, 0), ALU.subtract)
        else:  # ry
            eng.tensor_tensor(h(st, 0), h(st, 0), sw(B, 0), ALU.add)
            eng.tensor_tensor(h(st, 1), h(st, 1), sw(B, 1), ALU.add)
        return
    if kind == "rx":
        cross = [(0, 0, (1, 1), ALU.add), (1, 0, (0, 1), ALU.subtract),
                 (0, 1, (1, 0), ALU.add), (1, 1, (0, 0), ALU.subtract)]
    elif kind == "ry":
        cross = [(0, 0, (0, 1), ALU.subtract), (1, 0, (1, 1), ALU.subtract),
                 (0, 1, (0, 0), ALU.add), (1, 1, (1, 0), ALU.add)]
    else:  # rz
        cross = [(0, 0, (1, 0), ALU.add), (1, 0, (0, 0), ALU.subtract),
                 (0, 1, (1, 1), ALU.subtract), (1, 1, (0, 1), ALU.add)]
    for ri, k, (bri, bk), op in cross:
        o = st_ap(st, ri, {p: k})
        eng.tensor_tensor(o, o, st_ap(B, bri, {p: bk}), op)


def emit_crx(eng, xeng, nc_gp, st, B, co, si, epool, pc, pt, j):
    coff, cdims = state_dims({pc: 1})
    crep = epool.tile([128, 512], FP16, tag="exp5", name="crep")
    srep = epool.tile([128, 512], FP16, tag="exp5", name="srep")
    # value cos[b] at flat position b*32 + q  (q < 32 = 2^(5-pc) * 2^pc)
    xeng.copy(mkap(crep, 0, [[1, 512]]), mkap(co, j * NB, [[1, NB], [0, 32]]))
    xeng.copy(mkap(srep, 0, [[1, 512]]), mkap(si, j * NB, [[1, NB], [0, 32]]))
    for ri in (0, 1):
        v = mkap(st, ri * HALF + coff, cdims)
        eng.tensor_tensor(mkap(B, ri * HALF + coff, cdims), v,
                          mkap(srep, 0, [[1, 512]]), ALU.mult)
    for ri in (0, 1):
        v = mkap(st, ri * HALF + coff, cdims)
        eng.tensor_tensor(v, v, mkap(crep, 0, [[1, 512]]), ALU.mult)
    if pt == 0 and pc == 1:
        od = [[4, 256], [1, 2]]
        for ro, bi, op in ((0, 1, ALU.add), (1, 0, ALU.subtract)):
            eng.tensor_tensor(
                mkap(st, ro * HALF + coff, od), mkap(st, ro * HALF + coff, od),
                mkap(B, bi * HALF + coff + 1, [[4, 256], [-1, 2]]), op)
        return
    cross = [(0, 0, (1, 1), ALU.add), (1, 0, (0, 1), ALU.subtract),
             (0, 1, (1, 0), ALU.add), (1, 1, (0, 0), ALU.subtract)]
    for ri, k, (bri, bk), op in cross:
        o = st_ap(st, ri, {pc: 1, pt: k})
        eng.tensor_tensor(o, o, st_ap(B, bri, {pc: 1, pt: bk}), op)


def emit_ansatz(eng, xeng, nc_gp, st, B, co, si, epool, n_layers, sparse):
    for gi, (kind, loc, j) in enumerate(ansatz_gates(n_layers)):
        if kind == "crx":
            wc, wt = loc
            emit_crx(eng, xeng, nc_gp, st, B, co, si, epool, 5 - wc, 5 - wt, j)
        else:
            p = 5 - loc
            in_l0 = sparse and gi < 3 * NQ
            if in_l0 and p >= 1:
                sup = set(range(p))
                if kind == "rx":
                    emit_rx_first(eng, st, B, co, si, p, j, sup)
                else:
                    emit_rot_sp(eng, st, B, co, si, kind, p, j, sup)
            else:
                emit_rot_dense(eng, xeng, st, B, co, si, epool, kind, p, j)


def _split_multi_waits(nc):
    """This walrus build allows at most ONE sync-wait per instruction."""
    ctr = [0]
    for f in nc.m.functions:
        for b in f.blocks:
            new = []
            for inst in b.instructions:
                si = inst.sync_info
                if si is not None and len(si.on_wait) > 1:
                    waits = list(si.on_wait)
                    for w in waits[:-1]:
                        ctr[0] += 1
                        nop = mybir.InstNoOp(
                            name=f"wsplit-{ctr[0]}",
                            ins=[],
                            outs=[],
                            engine=inst.engine,
                            sync_info=mybir.SyncInfo(on_wait=[w], on_update=[]),
                        )
                        new.append(nop)
                    inst.sync_info = mybir.SyncInfo(
                        on_wait=[waits[-1]], on_update=list(si.on_update)
                    )
                new.append(inst)
            b.instructions = new


# ---------------------------------------------------------------- program
def build_program(split_waits=True):
    nc = bass.Bass()

    for v in (float(np.pi / 2), 1e-5, 0.25, float(0.25 + np.pi / 2)):
        t = nc.alloc_sbuf_tensor(f"const-f32-{v}", [128, 1], F32)
        nc.gpsimd.memset(t.ap(), v)
        nc.const_aps.aps[(F32, v)] = t.ap()
    nc.all_engine_barrier()

    # ---- dram I/O (per core) ----
    xs = nc.declare_dram_parameter("xs", [BPC, C_IN, T], BF16, isOutput=False)
    xp = nc.declare_dram_parameter("xp", [BPC, NC, CH * C_IN], FP16, isOutput=False)
    wfb = nc.declare_dram_parameter("wfb", [C_IN + 1, 128], BF16, isOutput=False)
    aw2 = nc.declare_dram_parameter("aw2", [128, 1], BF16, isOutput=False)
    ewb = nc.declare_dram_parameter("ewb", [C_IN + 1, D], BF16, isOutput=False)
    pjw = nc.declare_dram_parameter("pjw", [128, 120], BF16, isOutput=False)
    pjb = nc.declare_dram_parameter("pjb", [1, 60], BF16, isOutput=False)
    cf3 = nc.declare_dram_parameter("cf3", [NC, 3], FP16, isOutput=False)
    kbig = nc.declare_dram_parameter("kbig", [128, 18 * 128], FP16, isOutput=False)
    owb = nc.declare_dram_parameter("owb", [19, D], F32, isOutput=False)
    lng = nc.declare_dram_parameter("lng", [BPC, D], F32, isOutput=False)
    lnb = nc.declare_dram_parameter("lnb", [BPC, D], F32, isOutput=False)
    cw1 = nc.declare_dram_parameter("cw1", [128, 2 * D], F32, isOutput=False)
    cb1 = nc.declare_dram_parameter("cb1", [1, D], F32, isOutput=False)
    cw2 = nc.declare_dram_parameter("cw2", [128, 4], F32, isOutput=False)
    cb2 = nc.declare_dram_parameter("cb2", [1, 2], F32, isOutput=False)
    idn = nc.declare_dram_parameter("idn", [128, 128], F32, isOutput=False)
    idn16 = nc.declare_dram_parameter("idn16", [128, 128], FP16, isOutput=False)
    out = nc.declare_dram_parameter("out", [BPC, 2], F32, isOutput=True)

    with tile.TileContext(nc) as tc:
        with (
            tc.tile_pool(name="const", bufs=1) as cp,
            tc.tile_pool(name="xbuf", bufs=3) as xpool,
            tc.tile_pool(name="xpbuf", bufs=3) as xppool,
            tc.tile_pool(name="tanh", bufs=3) as thpool,
            tc.tile_pool(name="exp", bufs=12) as epool,
            tc.tile_pool(name="small", bufs=4) as sm,
            tc.tile_pool(name="ps_h", bufs=2, space="PSUM") as ps_h,
            tc.tile_pool(name="ps_s", bufs=2, space="PSUM") as ps_s,
            tc.tile_pool(name="ps_m", bufs=2, space="PSUM") as ps_m,
            tc.tile_pool(name="ps_t", bufs=2, space="PSUM") as ps_t,
        ):
            # ---------------- constants into SBUF ----------------
            def cload(name, dram, shape, dt=F32):
                t = cp.tile(shape, dt, tag=name, name=name)
                nc.sync.dma_start(out=t, in_=dram[:, :])
                return t

            wfb_s = cload("wfb", wfb, [C_IN + 1, 128], BF16)
            aw2_s = cload("aw2", aw2, [128, 1], BF16)

            # per-b input buffers + prefetch of the first batch elems before
            # the remaining (later-needed) constant loads hit the DMA queue
            x_sb = [xpool.tile([C_IN + 1, T], BF16, tag="x", name=f"xsb{i}") for i in range(3)]
            xp_sb = [xppool.tile([NC, CH * C_IN], FP16, tag="xp", name=f"xpsb{i}") for i in range(3)]
            xwt_sb = [xppool.tile([C_IN + 1, NC], BF16, tag="xwt", name=f"xwtsb{i}") for i in range(3)]
            for i in range(3):
                nc.vector.memset(x_sb[i][C_IN : C_IN + 1, :], 1.0)
                nc.vector.memset(xwt_sb[i][C_IN : C_IN + 1, :], 1.0)
            for b0 in range(2):
                nc.sync.dma_start(out=x_sb[b0][0:C_IN, :], in_=xs[b0, :, :])
                nc.sync.dma_start(out=xp_sb[b0], in_=xp[b0, :, :])

            idn16_s = cload("idn16", idn16, [128, 128], FP16)
            ewb_s = cload("ewb", ewb, [C_IN + 1, D], BF16)
            pjw_s = cload("pjw", pjw, [128, 120], BF16)
            pjb_s = cload("pjb", pjb, [1, 60], BF16)
            cf3_s = cload("cf3", cf3, [NC, 3], FP16)

            ones = cp.tile([1, 128], F32, tag="ones")
            nc.vector.memset(ones, 1.0)
            ones_bf = cp.tile([1, 128], BF16, tag="ones_bf")
            nc.vector.memset(ones_bf, 1.0)

            # per-b score tiles (double-buffered via pool tags)
            scb_t = [sm.tile([NC, CH], F32, tag="scb", name=f"scb{i}") for i in range(3)]
            wb_t = [sm.tile([NC, CH], FP16, tag="wb", name=f"wb{i}") for i in range(3)]

            # batched quantum state + scratch + compact trig tiles
            st_all = cp.tile([128, 2 * HALF], FP16, tag="stall")
            B_all = cp.tile([128, 2 * HALF], FP16, tag="Ball")
            co_all = cp.tile([128, 60 * NB], FP16, tag="coall")
            si_all = cp.tile([128, 60 * NB], FP16, tag="siall")

            # init state |0..0> for all (chunk, b)
            nc.vector.memset(st_all, 0.0)
            nc.vector.memset(mkap(st_all, 0, [[64, NB], [1, 1]]), 1.0)

            qfeat = cp.tile([BPC, 19], F32, tag="qfeat")
            nc.vector.memset(qfeat[:, 18:19], 1.0)
            zrow = cp.tile([1, 2 * HALF], F32, tag="zrow")
            Zt = cp.tile([128, NB], FP16, tag="Zt")
            Zb = cp.tile([BPC, 128], F32, tag="Zb")

            # ================= classical per-b =================
            for b in range(BPC):
                g, bb = b // 8, b % 8
                if bb == 0:
                    par_g = ps_t.tile([NC, 8 * 60], F32, tag="t")
                xb = x_sb[b % 3]
                xpb = xp_sb[b % 3]
                if b >= 2:
                    nc.sync.dma_start(out=xb[0:C_IN, :], in_=xs[b, :, :])
                    nc.sync.dma_start(out=xpb, in_=xp[b, :, :])

                scb = scb_t[b % 3]
                th = thpool.tile([128, T], BF16, tag="th")
                for blk in range(4):
                    hp = ps_h.tile([128, 512], F32, tag="hp")
                    nc.tensor.matmul(
                        hp,
                        wfb_s[:, :],
                        xb[:, blk * 512 : (blk + 1) * 512],
                        start=True,
                        stop=True,
                    )
                    nc.scalar.activation(
                        th[:, blk * 512 : (blk + 1) * 512], hp, AF.Tanh
                    )
                    sc = ps_s.tile([1, 512], F32, tag="sc")
                    nc.tensor.matmul(
                        sc,
                        aw2_s[:, :],
                        th[:, blk * 512 : (blk + 1) * 512],
                        start=True,
                        stop=True,
                    )
                    ssc = sm.tile([1, 512], F32, tag="ssc", name="ssc")
                    if blk % 2 == 0:
                        nc.vector.tensor_scalar_mul(ssc, sc, 1.0)
                    else:
                        nc.scalar.copy(ssc, sc)
                    src_ = ssc.rearrange("p (n k) -> p n k", n=32, k=CH)
                    nc.sync.dma_start(
                        out=scb[blk * 32 : (blk + 1) * 32, :], in_=src_
                    )

                # per-b softmax
                esc = sm.tile([NC, CH], F32, tag="esc", name="esc")
                nc.scalar.activation(esc, scb, AF.Exp)
                ssum = sm.tile([NC, 1], F32, tag="ssum")
                nc.vector.tensor_reduce(ssum, esc, AX.X, ALU.add)
                rsum = sm.tile([NC, 1], F32, tag="rsum")
                nc.vector.reciprocal(rsum, ssum)
                wb = wb_t[b % 3]
                nc.gpsimd.tensor_scalar_mul(wb, esc, rsum[:, 0:1])

                # chunk path
                xwtmp = sm.tile([NC, CH * C_IN], FP16, tag="xwtmp", name="xwtmp")
                wv = mkap(wb, 0, [[0, C_IN], [1, CH]])
                nc.vector.tensor_tensor(
                    mkap(xwtmp, 0, [[1, CH * C_IN]]),
                    mkap(xpb, 0, [[1, CH * C_IN]]),
                    wv,
                    ALU.mult,
                )
                xw = sm.tile([NC, C_IN], FP16, tag="xw")
                with nc.allow_low_precision("16-elem fp16 sums, tol 2e-2"):
                    nc.vector.tensor_reduce(
                        xw,
                        mkap(xwtmp, 0, [[CH, C_IN], [1, CH]]),
                        AX.X,
                        ALU.add,
                    )
                xwt_ps = ps_m.tile([C_IN, NC], FP16, tag="m")
                nc.tensor.transpose(xwt_ps, xw, idn16_s)
                xwt = xwt_sb[b % 3]
                nc.vector.tensor_copy(xwt[0:C_IN, :], xwt_ps)
                cht = [None, None]
                for h in range(2):
                    chp = ps_m.tile([128, NC], F32, tag="m")
                    nc.tensor.matmul(
                        chp,
                        ewb_s[:, h * 128 : (h + 1) * 128],
                        xwt,
                        start=True,
                        stop=True,
                    )
                    cht[h] = sm.tile([128, NC], BF16, tag=f"cht{h}", name=f"cht{h}")
                    nc.scalar.copy(cht[h], chp)
                par = par_g[:, bb * 60 : (bb + 1) * 60]
                nc.tensor.matmul(
                    par, cht[0], pjw_s[:, 0:60],
                    start=True, stop=False, skip_group_check=True,
                )
                nc.tensor.matmul(
                    par, cht[1], pjw_s[:, 60:120],
                    start=False, stop=False, skip_group_check=True,
                )
                nc.tensor.matmul(
                    par, ones_bf, pjb_s,
                    start=False, stop=True, skip_group_check=True,
                )

                if bb == 7:
                    # theta = sigmoid(z) = 0.5 + 0.5*tanh(z/2); Tanh+Sin share
                    # an act table set.
                    tg = sm.tile([NC, 8 * 60], FP16, tag="tg", name="tg")
                    nc.scalar.activation(tg, par_g, AF.Tanh, scale=0.5)
                    co_dst = mkap(co_all, g * 8, [[1, 8], [NB, 60]])
                    si_dst = mkap(si_all, g * 8, [[1, 8], [NB, 60]])
                    tg_v = mkap(tg, 0, [[60, 8], [1, 60]])
                    nc.scalar.activation(
                        co_dst, tg_v, AF.Sin,
                        bias=float(0.25 + np.pi / 2), scale=0.25,
                    )
                    nc.scalar.activation(
                        si_dst, tg_v, AF.Sin, bias=0.25, scale=0.25
                    )

            idn_s = cload("idn", idn, [128, 128])
            kbig_s = cload("kbig", kbig, [128, 18 * 128], FP16)
            owb_s = cload("owb", owb, [19, D])
            lng_s = cload("lng", lng, [BPC, D])
            lnb_s = cload("lnb", lnb, [BPC, D])
            cw1_s = cload("cw1", cw1, [128, 2 * D])
            cb1_s = cload("cb1", cb1, [1, D])
            cw2_s = cload("cw2", cw2, [128, 4])
            cb2_s = cload("cb2", cb2, [1, 2])

            # ================= batched quantum stage 1 =================
            emit_ansatz(nc.vector, nc.scalar, nc.gpsimd, st_all, B_all,
                        co_all, si_all, epool, 2, sparse=True)

            # ================= LCU -> z rows =================
            # zx = cr^T st_re + (-ci)^T st_im ; zy = ci^T st_re + cr^T st_im
            for h in range(2):
                zx_ps = ps_m.tile([1, 512], F32, tag="m")
                nc.tensor.matmul(zx_ps, cf3_s[:, 0:1],
                                 st_all[:, h * 512 : (h + 1) * 512],
                                 start=True, stop=False)
                nc.tensor.matmul(zx_ps, cf3_s[:, 2:3],
                                 st_all[:, HALF + h * 512 : HALF + (h + 1) * 512],
                                 start=False, stop=True)
                nc.vector.tensor_scalar_mul(zrow[:, h * 512 : (h + 1) * 512], zx_ps, 1.0)
            for h in range(2):
                zy_ps = ps_m.tile([1, 512], F32, tag="m")
                nc.tensor.matmul(zy_ps, cf3_s[:, 1:2],
                                 st_all[:, h * 512 : (h + 1) * 512],
                                 start=True, stop=False)
                nc.tensor.matmul(zy_ps, cf3_s[:, 0:1],
                                 st_all[:, HALF + h * 512 : HALF + (h + 1) * 512],
                                 start=False, stop=True)
                nc.vector.tensor_scalar_mul(
                    zrow[:, HALF + h * 512 : HALF + (h + 1) * 512], zy_ps, 1.0)

            # zrow [1, (ri,b,amp)] -> Zb [16 part=b, 128=(ri,amp)] via 2 DMAs
            for ri in (0, 1):
                src = bass.AP(
                    tensor=zrow.tensor, offset=zrow.offset + ri * HALF,
                    ap=[list(zrow.ap[0])] + [[DIM, NB], [1, DIM]],
                )
                nc.sync.dma_start(out=Zb[:, ri * DIM : (ri + 1) * DIM], in_=src)
            # Zt [128, 16] = Zb^T
            zt_ps = ps_m.tile([128, BPC], F32, tag="m")
            nc.tensor.transpose(zt_ps, Zb, idn_s[0:BPC, 0:BPC])
            nc.vector.tensor_copy(Zt, zt_ps)

            # n = z.z per b; qfeat_o = (z^T K_o z) / n
            nsum = sm.tile([BPC, 1], F32, tag="nsum")
            scr0 = sm.tile([BPC, 128], F32, tag="scr0", name="scr0")
            nc.vector.scalar_tensor_tensor(
                scr0, Zb, 1.0, Zb, ALU.mult, ALU.mult, accum_out=nsum
            )
            scr = [sm.tile([BPC, 128], F32, tag=f"scrk{i}", name=f"scrk{i}")
                   for i in range(4)]
            for blk in range(5):
                n0 = blk * 4
                nobs = min(4, 18 - n0)
                y_ps = ps_t.tile([BPC, 512], F32, tag="t")
                nc.tensor.matmul(
                    y_ps[:, 0 : nobs * 128],
                    Zt[:, :],
                    kbig_s[:, n0 * 128 : (n0 + nobs) * 128],
                    start=True, stop=True,
                )
                for o in range(nobs):
                    nc.vector.scalar_tensor_tensor(
                        scr[o % 4],
                        y_ps[:, o * 128 : (o + 1) * 128],
                        1.0,
                        Zb,
                        ALU.mult, ALU.mult,
                        accum_out=qfeat[:, n0 + o : n0 + o + 1],
                    )
            rn = sm.tile([BPC, 1], F32, tag="rn")
            nc.vector.reciprocal(rn, nsum)
            nc.vector.tensor_scalar_mul(qfeat[:, 0:18], qfeat[:, 0:18], rn)

            # ================= tail =================
            qfT_ps = ps_m.tile([19, BPC], F32, tag="m")
            nc.tensor.transpose(qfT_ps, qfeat, idn_s[0:BPC, 0:BPC])
            qfT = sm.tile([19, BPC], F32, tag="qfTs")
            nc.vector.tensor_copy(qfT, qfT_ps)
            o1 = ps_t.tile([BPC, D], F32, tag="t")
            nc.tensor.matmul(o1, qfT, owb_s, start=True, stop=True)

            stats = sm.tile([BPC, 6], F32, tag="stats")
            nc.vector.bn_stats(stats, o1)
            mv = sm.tile([BPC, 2], F32, tag="mv")
            nc.vector.bn_aggr(mv, stats)
            sdv = sm.tile([BPC, 1], F32, tag="sdv")
            nc.scalar.activation(sdv, mv[:, 1:2], AF.Sqrt, bias=1e-5)
            rstd = sm.tile([BPC, 1], F32, tag="rstd")
            nc.vector.reciprocal(rstd, sdv)
            ln1 = sm.tile([BPC, D], F32, tag="ln1")
            nc.vector.tensor_scalar(
                ln1, o1, mv[:, 0:1], rstd, ALU.subtract, ALU.mult
            )
            ln2 = sm.tile([BPC, D], F32, tag="ln2")
            nc.vector.tensor_tensor(ln2, ln1, lng_s, ALU.mult)
            nc.vector.tensor_tensor(ln2, ln2, lnb_s, ALU.add)

            # cls layer 1
            lnT = [None, None]
            for h in range(2):
                lnT_ps = ps_m.tile([128, BPC], F32, tag="m")
                nc.tensor.transpose(
                    lnT_ps, ln2[:, h * 128 : (h + 1) * 128], idn_s[0:BPC, 0:BPC]
                )
                lnT[h] = sm.tile([128, BPC], F32, tag=f"lnT{h}", name=f"lnT{h}")
                nc.vector.tensor_copy(lnT[h], lnT_ps)
            h2p = ps_t.tile([BPC, D], F32, tag="t")
            nc.tensor.matmul(h2p, lnT[0], cw1_s[:, 0:D], start=True, stop=False)
            nc.tensor.matmul(
                h2p, lnT[1], cw1_s[:, D : 2 * D], start=False, stop=False
            )
            nc.tensor.matmul(
                h2p, ones[:, 0:BPC], cb1_s, start=False, stop=True
            )
            h2 = sm.tile([BPC, D], F32, tag="h2")
            nc.scalar.activation(h2, h2p, AF.Relu)

            # cls layer 2
            h2T = [None, None]
            for h in range(2):
                h2T_ps = ps_m.tile([128, BPC], F32, tag="m")
                nc.tensor.transpose(
                    h2T_ps, h2[:, h * 128 : (h + 1) * 128], idn_s[0:BPC, 0:BPC]
                )
                h2T[h] = sm.tile([128, BPC], F32, tag=f"h2T{h}", name=f"h2T{h}")
                nc.vector.tensor_copy(h2T[h], h2T_ps)
            lg = ps_t.tile([BPC, 2], F32, tag="t")
            nc.tensor.matmul(lg, h2T[0], cw2_s[:, 0:2], start=True, stop=False)
            nc.tensor.matmul(lg, h2T[1], cw2_s[:, 2:4], start=False, stop=False)
            nc.tensor.matmul(lg, ones[:, 0:BPC], cb2_s, start=False, stop=True)
            lgs = sm.tile([BPC, 2], F32, tag="lgs")
            nc.vector.tensor_copy(lgs, lg)
            nc.sync.dma_start(out=out[:, :], in_=lgs)

    if split_waits:
        _split_multi_waits(nc)
    return nc


_NC_CACHE = {}


def _get_program():
    if "nc" not in _NC_CACHE:
        _NC_CACHE["nc"] = build_program()
    return _NC_CACHE["nc"]


# ------------------------------------------------- host-side qff K matrices
def _np_rx(t):
    c, s = np.cos(t / 2), np.sin(t / 2)
    return np.array([[c, -1j * s], [-1j * s, c]])


def _np_ry(t):
    c, s = np.cos(t / 2), np.sin(t / 2)
    return np.array([[c, -s], [s, c]], complex)


def _np_rz(t):
    e = np.exp(-0.5j * t)
    return np.array([[e, 0], [0, np.conj(e)]])


def _np_apply_1q(state, gate, wire):
    # state [B, 64]; wire 0 = MSB
    dl, dr = 2 ** wire, 2 ** (NQ - 1 - wire)
    s = state.reshape(-1, dl, 2, dr)
    s = np.einsum('kj,bljr->blkr', gate, s)
    return s.reshape(-1, DIM)


def _np_apply_crx(state, theta, control, target):
    s = state.reshape((-1,) + (2,) * NQ)
    s = np.moveaxis(s, (control + 1, target + 1), (1, 2))
    g = _np_rx(theta)
    s1 = np.einsum('kj,bj...->bk...', g, s[:, 1])
    s = np.concatenate([s[:, :1], s1[:, None]], axis=1)
    s = np.moveaxis(s, (1, 2), (control + 1, target + 1))
    return s.reshape(-1, DIM)


def _np_ansatz(state, params, n_layers):
    idx = 0
    for _ in range(n_layers):
        for i in range(NQ):
            state = _np_apply_1q(state, _np_rx(params[idx]), i)
            state = _np_apply_1q(state, _np_ry(params[idx + 1]), i)
            state = _np_apply_1q(state, _np_rz(params[idx + 2]), i)
            idx += 3
        for i in range(NQ):
            state = _np_apply_crx(state, params[idx], i, (i + 1) % NQ)
            idx += 1
        for i in range(NQ - 1, -1, -1):
            state = _np_apply_crx(state, params[idx], i, (i - 1) % NQ)
            idx += 1
    return state


def _qff_k_matrices(qff_params):
    """Kbig [128, 18*128]: real quadratic forms for U^H O_o U, o = X0..X5,Y0..Y5,Z0..Z5."""
    U = _np_ansatz(np.eye(DIM, dtype=complex), np.asarray(qff_params, np.float64), 1)
    U = U.T  # rows of _np_ansatz output are evolved basis states -> columns of U
    PX = np.array([[0, 1], [1, 0]], complex)
    PY = np.array([[0, -1j], [1j, 0]])
    PZ = np.array([[1, 0], [0, -1]], complex)
    I2 = np.eye(2)
    blocks = []
    for P in (PX, PY, PZ):
        for i in range(NQ):
            O = np.array([[1.0 + 0j]])
            for w in range(NQ):
                O = np.kron(O, P if w == i else I2)
            M = U.conj().T @ O @ U
            A, Bm = M.real, M.imag
            K = np.block([[A, -Bm], [Bm, A]])
            blocks.append(K)
    return np.concatenate(blocks, axis=1).astype(np.float32)


def host_prep(inputs):
    """Host-side parameter folding -> per-core input maps."""
    f32 = np.float32
    x = np.asarray(inputs["x"], f32)
    emb_w = np.asarray(inputs["emb_w"], np.float64)
    emb_b = np.asarray(inputs["emb_b"], np.float64)
    att_w1 = np.asarray(inputs["att_w1"], np.float64)
    att_b1 = np.asarray(inputs["att_b1"], np.float64)

    bf16 = ml_dtypes.bfloat16
    wfold = (emb_w @ att_w1).astype(f32)  # [64, 128]
    bfold = (emb_b @ att_w1 + att_b1).astype(f32)  # [128]
    wfb = np.concatenate([wfold, bfold[None, :]], 0).astype(bf16)  # [65, 128]

    ewb = np.concatenate(
        [emb_w.astype(f32), emb_b.astype(f32)[None, :]], 0
    ).astype(bf16)  # [65, 256]

    pw = np.asarray(inputs["proj_w"], f32)  # [256, 60]
    pjw = np.concatenate([pw[0:128, :], pw[128:256, :]], 1).astype(bf16)  # [128, 120]

    cr = np.asarray(inputs["mix_re"], np.float64)
    ci = np.asarray(inputs["mix_im"], np.float64)
    den = np.sqrt(cr * cr + ci * ci).sum() + 1e-8
    cf3 = np.stack([cr / den, ci / den, -ci / den], 1).astype(np.float16)  # [128, 3]

    kbig = _qff_k_matrices(inputs["qff_params"]).astype(np.float16)  # [128, 2304]

    owb = np.concatenate(
        [np.asarray(inputs["out_w"], f32), np.asarray(inputs["out_b"], f32)[None, :]],
        0,
    )  # [19, 256]
    lng = np.broadcast_to(np.asarray(inputs["ln_g"], f32), (BPC, D)).copy()
    lnb = np.broadcast_to(np.asarray(inputs["ln_b"], f32), (BPC, D)).copy()
    w1 = np.asarray(inputs["cls_w1"], f32)
    cw1 = np.concatenate([w1[0:128, :], w1[128:256, :]], 1)  # [128, 512]
    cb1 = np.asarray(inputs["cls_b1"], f32)[None, :]
    w2 = np.asarray(inputs["cls_w2"], f32)
    cw2 = np.concatenate([w2[0:128, :], w2[128:256, :]], 1)  # [128, 4]
    cb2 = np.asarray(inputs["cls_b2"], f32)[None, :]
    idn = np.eye(128, dtype=f32)
    idn16 = np.eye(128, dtype=np.float16)
    pjb = np.asarray(inputs["proj_b"], f32)[None, :]

    shared = dict(
        wfb=wfb, aw2=np.asarray(inputs["att_w2"], f32).astype(bf16), ewb=ewb,
        pjw=pjw, pjb=pjb.astype(bf16), cf3=cf3, kbig=kbig, owb=owb, lng=lng,
        lnb=lnb, cw1=cw1, cb1=cb1, cw2=cw2, cb2=cb2, idn=idn, idn16=idn16,
    )

    in_maps = []
    for c in range(N_CORES):
        xc = x[c * BPC : (c + 1) * BPC]  # [16, 64, 2048]
        # xperm[b, nc, c*16+k] = x[b, c, nc*16+k]
        xp_c = np.ascontiguousarray(
            xc.reshape(BPC, C_IN, NC, CH).transpose(0, 2, 1, 3).reshape(
                BPC, NC, CH * C_IN
            )
        )
        m = dict(shared)
        m["xs"] = np.ascontiguousarray(xc).astype(bf16)
        m["xp"] = xp_c.astype(np.float16)
        in_maps.append(m)
    return in_maps


def kernel(**inputs):
    nc = _get_program()
    in_maps = host_prep(inputs)
    res = run_bass_kernel_spmd(nc, in_maps, core_ids=list(range(N_CORES)))
    outs = [res.results[c]["out"] for c in range(N_CORES)]
    return np.concatenate(outs, 0).astype(np.float32)


if __name__ == "__main__":
    nc = build_program()
    print("program built ok")


# revision 29
# speedup vs baseline: 1.0551x; 1.0058x over previous
"""Trainium2 Bass kernel for nn_ClassicalQuantumAttention.

Data-parallel over batch: 128 batch elems -> 16 per NeuronCore x 8 cores.

v2: batched quantum stage. All 16 batch elems' statevectors live in ONE
tile st_all [128 part = chunks, 2048 free] with free addr
  ri*1024 + b*64 + amp     (ri = re/im, b in 0..15, amp 6 bits, wire w <-> bit 5-w)
Gates become ~8 large tensor_tensor ops (vs ~80 small per-b ops): per-(chunk,b)
cos/sin factors broadcast via stride-0 APs from compact tiles co_all/si_all
[128, 60*16] (layout j*16+b). crx gates with ctrl bit < 5 need small replicated
cos/sin tiles (expanded on gpsimd).

Tail: the shared-param qff ansatz + expvals + normalization collapse into 18
host-precomputed real quadratic forms K_o = [[A,-B],[B,A]] (M_o = U^H O_o U):
qfeat_o(b) = z^T K_o z / (z^T z), computed with PE matmuls + STT-accum reductions.

Classical path in bf16 (host-converted), quantum state in fp16.
"""

import numpy as np
import ml_dtypes
import sys

for _p in ("/opt/trn_rl_repo",):
    if _p not in sys.path:
        sys.path.insert(0, _p)

import concourse.bass as bass
import concourse.tile as tile
from concourse import mybir
from concourse.bass_utils import run_bass_kernel_spmd

F32 = mybir.dt.float32
BF16 = mybir.dt.bfloat16
FP16 = mybir.dt.float16
ALU = mybir.AluOpType
AF = mybir.ActivationFunctionType
AX = mybir.AxisListType

N_CORES = 8
B_TOT = 128
BPC = B_TOT // N_CORES  # 16 batch elems per core
C_IN = 64
T = 2048
D = 256
CH = 16
NC = T // CH  # 128 chunks
NQ = 6
DIM = 64  # 2**6 amplitudes
NB = 16  # batch elems per core (= BPC)
HALF = NB * DIM  # 1024 floats per ri half


# ---------------------------------------------------------------- gate list
def ansatz_gates(n_layers):
    """[(kind, wire-or-(ctrl,tgt), param_idx)] matching reference _ansatz."""
    gates = []
    idx = 0
    for _ in range(n_layers):
        for i in range(NQ):
            gates.append(("rx", i, idx))
            gates.append(("ry", i, idx + 1))
            gates.append(("rz", i, idx + 2))
            idx += 3
        for i in range(NQ):
            gates.append(("crx", (i, (i + 1) % NQ), idx))
            idx += 1
        for i in range(NQ - 1, -1, -1):
            gates.append(("crx", (i, (i - 1) % NQ), idx))
            idx += 1
    return gates


# ------------------------------------------------------------- AP helpers
def state_dims(fixed):
    """(amp_offset, dims) over (b x free amp bits) for one ri half.

    fixed: {bitpos: 0/1}. The b dim (stride 64, count 16) merges with the
    top amp run when that run spans 64; all ansatz cases yield <=2 free dims.
    """
    runs = []
    run = None
    offset = 0
    for p in range(5, -1, -1):
        if p in fixed:
            if run is not None:
                runs.append(run)
                run = None
            offset += fixed[p] << p
        else:
            if run is None:
                run = [1 << p, 2]
            else:
                run = [1 << p, run[1] * 2]
    if run is not None:
        runs.append(run)
    if runs and runs[0][0] * runs[0][1] == 64:
        dims = [[runs[0][0], NB * runs[0][1]]] + runs[1:]
    else:
        dims = [[64, NB]] + runs
    assert len(dims) <= 2, (fixed, dims)
    return offset, dims


def mkap(t, off, dims):
    return bass.AP(tensor=t.tensor, offset=t.offset + off, ap=[list(t.ap[0])] + dims)


def st_ap(t, ri, fixed):
    off, dims = state_dims(fixed)
    return mkap(t, ri * HALF + off, dims)


def cs_ap(t, j, dims):
    """Compact co/si broadcast view matching a merged state view's dims."""
    if len(dims) == 1:
        r0 = dims[0][1] // NB
        return mkap(t, j * NB, [[1, NB], [0, r0]])
    (s0, c0), (s1, c1) = dims
    assert c0 // NB == 1, "needs rep tile"
    return mkap(t, j * NB, [[1, NB], [0, c1]])


def rep_ap(t, dims):
    """Rep-tile (layout b*R+u) view matching state dims."""
    if len(dims) == 1:
        return mkap(t, 0, [[1, dims[0][1]]])
    (s0, c0), (s1, c1) = dims
    return mkap(t, 0, [[1, c0], [0, c1]])


# ------------------------------------------------------------ gate emitters
# Dense gates materialize per-gate cos/sin into contiguous fp16 tiles via one
# ACT copy each (stride-0 source reads); every DVE op then has a packed
# unit-stride last dim -> 2x DVE mode. Sparse layer-1 gates (wires 0-4) keep
# the compact broadcast form (regions too small/scattered to benefit).


def emit_rot_sp(eng, st, B, co, si, kind, p, j, sup):
    """Sparse layer-1 rotation (support-restricted, compact broadcast cs)."""
    fx = {q: 0 for q in sup}
    hoff, hdims = state_dims(fx)
    for ri in (0, 1):
        v = mkap(st, ri * HALF + hoff, hdims)
        eng.tensor_tensor(mkap(B, ri * HALF + hoff, hdims), v, cs_ap(si, j, hdims),
                          ALU.mult)
    for ri in (0, 1):
        v = mkap(st, ri * HALF + hoff, hdims)
        eng.tensor_tensor(v, v, cs_ap(co, j, hdims), ALU.mult)
    if kind == "rx":
        cross = [(0, 0, (1, 1), ALU.add), (1, 0, (0, 1), ALU.subtract),
                 (0, 1, (1, 0), ALU.add), (1, 1, (0, 0), ALU.subtract)]
    elif kind == "ry":
        cross = [(0, 0, (0, 1), ALU.subtract), (1, 0, (1, 1), ALU.subtract),
                 (0, 1, (0, 0), ALU.add), (1, 1, (1, 0), ALU.add)]
    else:  # rz
        cross = [(0, 0, (1, 0), ALU.add), (1, 0, (0, 0), ALU.subtract),
                 (0, 1, (1, 1), ALU.subtract), (1, 1, (0, 1), ALU.add)]
    for ri, k, (bri, bk), op in cross:
        o = st_ap(st, ri, {**fx, p: k})
        eng.tensor_tensor(o, o, st_ap(B, bri, {**fx, p: bk}), op)


def emit_rx_first(eng, st, B, co, si, p, j, sup):
    """RX on bit p when all bits <= p are zero (sparse layer-1 start)."""
    fx = {q: 0 for q in sup}
    off0, d0 = state_dims({**fx, p: 0})
    for ri in (0, 1):
        v = mkap(st, ri * HALF + off0, d0)
        eng.tensor_tensor(mkap(B, ri * HALF + off0, d0), v, cs_ap(si, j, d0),
                          ALU.mult)
    for ri in (0, 1):
        v = mkap(st, ri * HALF + off0, d0)
        eng.tensor_tensor(v, v, cs_ap(co, j, d0), ALU.mult)
    off1, d1 = state_dims({**fx, p: 1})
    eng.tensor_copy(mkap(st, 0 * HALF + off1, d1), mkap(B, 1 * HALF + off0, d0))
    eng.tensor_scalar_mul(mkap(st, 1 * HALF + off1, d1),
                          mkap(B, 0 * HALF + off0, d0), -1.0)


def emit_rot_dense(eng, xeng, st, B, co, si, epool, kind, p, j):
    """Dense rotation: materialized cos/sin, merged both-ri B and *c ops."""
    cfull = epool.tile([128, HALF], FP16, tag="exp", name="cfull")
    sfull = epool.tile([128, HALF], FP16, tag="exp", name="sfull")
    xeng.copy(mkap(cfull, 0, [[1, HALF]]), mkap(co, j * NB, [[1, NB], [0, DIM]]))
    if p == 0 and kind in ("ry", "rz"):
        # sigma-signed sin: s[b] * (-1)^(amp&1)
        xeng.copy(mkap(sfull, 0, [[2, HALF // 2]]),
                  mkap(si, j * NB, [[1, NB], [0, 32]]))
        xeng.mul(mkap(sfull, 1, [[2, HALF // 2]]),
                 mkap(si, j * NB, [[1, NB], [0, 32]]), -1.0)
    else:
        xeng.copy(mkap(sfull, 0, [[1, HALF]]), mkap(si, j * NB, [[1, NB], [0, DIM]]))
    both = lambda t: mkap(t, 0, [[1, 2 * HALF]])
    bc2 = lambda t: mkap(t, 0, [[0, 2], [1, HALF]])
    eng.tensor_tensor(both(B), both(st), bc2(sfull), ALU.mult)
    eng.tensor_tensor(both(st), both(st), bc2(cfull), ALU.mult)
    if p == 0:
        h = lambda t, ri: mkap(t, ri * HALF, [[1, HALF]])
        sw = lambda t, ri: mkap(t, ri * HALF + 1, [[2, HALF // 2], [-1, 2]])
        if kind == "rz":
            eng.tensor_tensor(h(st, 0), h(st, 0), h(B, 1), ALU.add)
            eng.tensor_tensor(h(st, 1), h(st, 1), h(B, 0), ALU.subtract)
        elif kind == "rx":
            eng.tensor_tensor(h(st, 0), h(st, 0), sw(B, 1), ALU.add)
            eng.tensor_tensor(h(st, 1), h(st, 1), sw(B, 0), ALU.subtract)
        else:  # ry
            eng.tensor_tensor(h(st, 0), h(st, 0), sw(B, 0), ALU.add)
            eng.tensor_tensor(h(st, 1), h(st, 1), sw(B, 1), ALU.add)
        return
    if kind == "rx":
        cross = [(0, 0, (1, 1), ALU.add), (1, 0, (0, 1), ALU.subtract),
                 (0, 1, (1, 0), ALU.add), (1, 1, (0, 0), ALU.subtract)]
    elif kind == "ry":
        cross = [(0, 0, (0, 1), ALU.subtract), (1, 0, (1, 1), ALU.subtract),
                 (0, 1, (0, 0), ALU.add), (1, 1, (1, 0), ALU.add)]
    else:  # rz
        cross = [(0, 0, (1, 0), ALU.add), (1, 0, (0, 0), ALU.subtract),
                 (0, 1, (1, 1), ALU.subtract), (1, 1, (0, 1), ALU.add)]
    for ri, k, (bri, bk), op in cross:
        o = st_ap(st, ri, {p: k})
        eng.tensor_tensor(o, o, st_ap(B, bri, {p: bk}), op)


def emit_crx(eng, xeng, nc_gp, st, B, co, si, epool, pc, pt, j):
    coff, cdims = state_dims({pc: 1})
    crep = epool.tile([128, 512], FP16, tag="exp5", name="crep")
    srep = epool.tile([128, 512], FP16, tag="exp5", name="srep")
    # value cos[b] at flat position b*32 + q  (q < 32 = 2^(5-pc) * 2^pc)
    xeng.copy(mkap(crep, 0, [[1, 512]]), mkap(co, j * NB, [[1, NB], [0, 32]]))
    xeng.copy(mkap(srep, 0, [[1, 512]]), mkap(si, j * NB, [[1, NB], [0, 32]]))
    for ri in (0, 1):
        v = mkap(st, ri * HALF + coff, cdims)
        eng.tensor_tensor(mkap(B, ri * HALF + coff, cdims), v,
                          mkap(srep, 0, [[1, 512]]), ALU.mult)
    for ri in (0, 1):
        v = mkap(st, ri * HALF + coff, cdims)
        eng.tensor_tensor(v, v, mkap(crep, 0, [[1, 512]]), ALU.mult)
    if pt == 0 and pc == 1:
        od = [[4, 256], [1, 2]]
        for ro, bi, op in ((0, 1, ALU.add), (1, 0, ALU.subtract)):
            eng.tensor_tensor(
                mkap(st, ro * HALF + coff, od), mkap(st, ro * HALF + coff, od),
                mkap(B, bi * HALF + coff + 1, [[4, 256], [-1, 2]]), op)
        return
    cross = [(0, 0, (1, 1), ALU.add), (1, 0, (0, 1), ALU.subtract),
             (0, 1, (1, 0), ALU.add), (1, 1, (0, 0), ALU.subtract)]
    for ri, k, (bri, bk), op in cross:
        o = st_ap(st, ri, {pc: 1, pt: k})
        eng.tensor_tensor(o, o, st_ap(B, bri, {pc: 1, pt: bk}), op)


def emit_ansatz(eng, xeng, nc_gp, st, B, co, si, epool, n_layers, sparse):
    for gi, (kind, loc, j) in enumerate(ansatz_gates(n_layers)):
        if kind == "crx":
            wc, wt = loc
            emit_crx(eng, xeng, nc_gp, st, B, co, si, epool, 5 - wc, 5 - wt, j)
        else:
            p = 5 - loc
            in_l0 = sparse and gi < 3 * NQ
            if in_l0 and p >= 1:
                sup = set(range(p))
                if kind == "rx":
                    emit_rx_first(eng, st, B, co, si, p, j, sup)
                else:
                    emit_rot_sp(eng, st, B, co, si, kind, p, j, sup)
            else:
                emit_rot_dense(eng, xeng, st, B, co, si, epool, kind, p, j)


def _split_multi_waits(nc):
    """This walrus build allows at most ONE sync-wait per instruction."""
    ctr = [0]
    for f in nc.m.functions:
        for b in f.blocks:
            new = []
            for inst in b.instructions:
                si = inst.sync_info
                if si is not None and len(si.on_wait) > 1:
                    waits = list(si.on_wait)
                    for w in waits[:-1]:
                        ctr[0] += 1
                        nop = mybir.InstNoOp(
                            name=f"wsplit-{ctr[0]}",
                            ins=[],
                            outs=[],
                            engine=inst.engine,
                            sync_info=mybir.SyncInfo(on_wait=[w], on_update=[]),
                        )
                        new.append(nop)
                    inst.sync_info = mybir.SyncInfo(
                        on_wait=[waits[-1]], on_update=list(si.on_update)
                    )
                new.append(inst)
            b.instructions = new


# ---------------------------------------------------------------- program
def build_program(split_waits=True):
    nc = bass.Bass()

    for v in (float(np.pi / 2), 1e-5, 0.25, float(0.25 + np.pi / 2)):
        t = nc.alloc_sbuf_tensor(f"const-f32-{v}", [128, 1], F32)
        nc.gpsimd.memset(t.ap(), v)
        nc.const_aps.aps[(F32, v)] = t.ap()
    nc.all_engine_barrier()

    # ---- dram I/O (per core) ----
    xs = nc.declare_dram_parameter("xs", [BPC, C_IN, T], BF16, isOutput=False)
    xp = nc.declare_dram_parameter("xp", [BPC, NC, CH * C_IN], FP16, isOutput=False)
    wfb = nc.declare_dram_parameter("wfb", [C_IN + 1, 128], BF16, isOutput=False)
    aw2 = nc.declare_dram_parameter("aw2", [128, 1], BF16, isOutput=False)
    ewb = nc.declare_dram_parameter("ewb", [C_IN + 1, D], BF16, isOutput=False)
    pjw = nc.declare_dram_parameter("pjw", [128, 120], BF16, isOutput=False)
    pjb = nc.declare_dram_parameter("pjb", [1, 60], BF16, isOutput=False)
    cf3 = nc.declare_dram_parameter("cf3", [NC, 3], FP16, isOutput=False)
    kbig = nc.declare_dram_parameter("kbig", [128, 18 * 128], FP16, isOutput=False)
    owb = nc.declare_dram_parameter("owb", [19, D], F32, isOutput=False)
    lng = nc.declare_dram_parameter("lng", [BPC, D], F32, isOutput=False)
    lnb = nc.declare_dram_parameter("lnb", [BPC, D], F32, isOutput=False)
    cw1 = nc.declare_dram_parameter("cw1", [128, 2 * D], F32, isOutput=False)
    cb1 = nc.declare_dram_parameter("cb1", [1, D], F32, isOutput=False)
    cw2 = nc.declare_dram_parameter("cw2", [128, 4], F32, isOutput=False)
    cb2 = nc.declare_dram_parameter("cb2", [1, 2], F32, isOutput=False)
    idn = nc.declare_dram_parameter("idn", [128, 128], F32, isOutput=False)
    idn16 = nc.declare_dram_parameter("idn16", [128, 128], FP16, isOutput=False)
    out = nc.declare_dram_parameter("out", [BPC, 2], F32, isOutput=True)

    with tile.TileContext(nc) as tc:
        with (
            tc.tile_pool(name="const", bufs=1) as cp,
            tc.tile_pool(name="xbuf", bufs=3) as xpool,
            tc.tile_pool(name="xpbuf", bufs=3) as xppool,
            tc.tile_pool(name="tanh", bufs=3) as thpool,
            tc.tile_pool(name="exp", bufs=12) as epool,
            tc.tile_pool(name="small", bufs=4) as sm,
            tc.tile_pool(name="ps_h", bufs=2, space="PSUM") as ps_h,
            tc.tile_pool(name="ps_s", bufs=2, space="PSUM") as ps_s,
            tc.tile_pool(name="ps_m", bufs=2, space="PSUM") as ps_m,
            tc.tile_pool(name="ps_t", bufs=2, space="PSUM") as ps_t,
        ):
            # ---------------- constants into SBUF ----------------
            def cload(name, dram, shape, dt=F32):
                t = cp.tile(shape, dt, tag=name, name=name)
                nc.sync.dma_start(out=t, in_=dram[:, :])
                return t

            wfb_s = cload("wfb", wfb, [C_IN + 1, 128], BF16)
            aw2_s = cload("aw2", aw2, [128, 1], BF16)

            # per-b input buffers + prefetch of the first batch elems before
            # the remaining (later-needed) constant loads hit the DMA queue
            x_sb = [xpool.tile([C_IN + 1, T], BF16, tag="x", name=f"xsb{i}") for i in range(3)]
            xp_sb = [xppool.tile([NC, CH * C_IN], FP16, tag="xp", name=f"xpsb{i}") for i in range(3)]
            xwt_sb = [xppool.tile([C_IN + 1, NC], BF16, tag="xwt", name=f"xwtsb{i}") for i in range(3)]
            for i in range(3):
                nc.vector.memset(x_sb[i][C_IN : C_IN + 1, :], 1.0)
                nc.vector.memset(xwt_sb[i][C_IN : C_IN + 1, :], 1.0)
            for b0 in range(2):
                nc.sync.dma_start(out=x_sb[b0][0:C_IN, :], in_=xs[b0, :, :])
                nc.sync.dma_start(out=xp_sb[b0], in_=xp[b0, :, :])

            idn16_s = cload("idn16", idn16, [128, 128], FP16)
            ewb_s = cload("ewb", ewb, [C_IN + 1, D], BF16)
            pjw_s = cload("pjw", pjw, [128, 120], BF16)
            pjb_s = cload("pjb", pjb, [1, 60], BF16)
            cf3_s = cload("cf3", cf3, [NC, 3], FP16)

            ones = cp.tile([1, 128], F32, tag="ones")
            nc.vector.memset(ones, 1.0)
            ones_bf = cp.tile([1, 128], BF16, tag="ones_bf")
            nc.vector.memset(ones_bf, 1.0)

            # per-b score tiles (double-buffered via pool tags)
            scb_t = [sm.tile([NC, CH], F32, tag="scb", name=f"scb{i}") for i in range(3)]
            wb_t = [sm.tile([NC, CH], FP16, tag="wb", name=f"wb{i}") for i in range(3)]

            # batched quantum state + scratch + compact trig tiles
            st_all = cp.tile([128, 2 * HALF], FP16, tag="stall")
            B_all = cp.tile([128, 2 * HALF], FP16, tag="Ball")
            co_all = cp.tile([128, 60 * NB], FP16, tag="coall")
            si_all = cp.tile([128, 60 * NB], FP16, tag="siall")

            # init state |0..0> for all (chunk, b)
            nc.vector.memset(st_all, 0.0)
            nc.vector.memset(mkap(st_all, 0, [[64, NB], [1, 1]]), 1.0)

            qfeat = cp.tile([BPC, 19], F32, tag="qfeat")
            nc.vector.memset(qfeat[:, 18:19], 1.0)
            zrow = cp.tile([1, 2 * HALF], F32, tag="zrow")
            Zt = cp.tile([128, NB], FP16, tag="Zt")
            Zb = cp.tile([BPC, 128], F32, tag="Zb")

            # ================= classical per-b =================
            for b in range(BPC):
                g, bb = b // 8, b % 8
                if bb == 0:
                    par_g = ps_t.tile([NC, 8 * 60], F32, tag="t")
                xb = x_sb[b % 3]
                xpb = xp_sb[b % 3]
                if b >= 2:
                    nc.sync.dma_start(out=xb[0:C_IN, :], in_=xs[b, :, :])
                    nc.sync.dma_start(out=xpb, in_=xp[b, :, :])

                scb = scb_t[b % 3]
                th = thpool.tile([128, T], BF16, tag="th")
                for blk in range(4):
                    hp = ps_h.tile([128, 512], F32, tag="hp")
                    nc.tensor.matmul(
                        hp,
                        wfb_s[:, :],
                        xb[:, blk * 512 : (blk + 1) * 512],
                        start=True,
                        stop=True,
                    )
                    nc.scalar.activation(
                        th[:, blk * 512 : (blk + 1) * 512], hp, AF.Tanh
                    )
                    sc = ps_s.tile([1, 512], F32, tag="sc")
                    nc.tensor.matmul(
                        sc,
                        aw2_s[:, :],
                        th[:, blk * 512 : (blk + 1) * 512],
                        start=True,
                        stop=True,
                    )
                    if blk == 0:
                        sscb = sm.tile([1, 4 * 512], F32, tag="sscb", name="sscb")
                    if blk % 2 == 0:
                        nc.vector.tensor_scalar_mul(
                            sscb[:, blk * 512 : (blk + 1) * 512], sc, 1.0)
                    else:
                        nc.scalar.copy(sscb[:, blk * 512 : (blk + 1) * 512], sc)
                if True:
                    # one scatter for all 4 blocks: [1,(128 nc,16)] -> [128,16]
                    src_ = sscb.rearrange("p (n k) -> p n k", n=NC, k=CH)
                    nc.sync.dma_start(out=scb, in_=src_)

                # per-b softmax
                esc = sm.tile([NC, CH], F32, tag="esc", name="esc")
                nc.scalar.activation(esc, scb, AF.Exp)
                ssum = sm.tile([NC, 1], F32, tag="ssum")
                nc.vector.tensor_reduce(ssum, esc, AX.X, ALU.add)
                rsum = sm.tile([NC, 1], F32, tag="rsum")
                nc.vector.reciprocal(rsum, ssum)
                wb = wb_t[b % 3]
                nc.gpsimd.tensor_scalar_mul(wb, esc, rsum[:, 0:1])

                # chunk path
                xwtmp = sm.tile([NC, CH * C_IN], FP16, tag="xwtmp", name="xwtmp")
                wv = mkap(wb, 0, [[0, C_IN], [1, CH]])
                nc.vector.tensor_tensor(
                    mkap(xwtmp, 0, [[1, CH * C_IN]]),
                    mkap(xpb, 0, [[1, CH * C_IN]]),
                    wv,
                    ALU.mult,
                )
                xw = sm.tile([NC, C_IN], FP16, tag="xw")
                with nc.allow_low_precision("16-elem fp16 sums, tol 2e-2"):
                    nc.vector.tensor_reduce(
                        xw,
                        mkap(xwtmp, 0, [[CH, C_IN], [1, CH]]),
                        AX.X,
                        ALU.add,
                    )
                xwt_ps = ps_m.tile([C_IN, NC], FP16, tag="m")
                nc.tensor.transpose(xwt_ps, xw, idn16_s)
                xwt = xwt_sb[b % 3]
                nc.vector.tensor_copy(xwt[0:C_IN, :], xwt_ps)
                cht = [None, None]
                for h in range(2):
                    chp = ps_m.tile([128, NC], F32, tag="m")
                    nc.tensor.matmul(
                        chp,
                        ewb_s[:, h * 128 : (h + 1) * 128],
                        xwt,
                        start=True,
                        stop=True,
                    )
                    cht[h] = sm.tile([128, NC], BF16, tag=f"cht{h}", name=f"cht{h}")
                    nc.scalar.copy(cht[h], chp)
                par = par_g[:, bb * 60 : (bb + 1) * 60]
                nc.tensor.matmul(
                    par, cht[0], pjw_s[:, 0:60],
                    start=True, stop=False, skip_group_check=True,
                )
                nc.tensor.matmul(
                    par, cht[1], pjw_s[:, 60:120],
                    start=False, stop=False, skip_group_check=True,
                )
                nc.tensor.matmul(
                    par, ones_bf, pjb_s,
                    start=False, stop=True, skip_group_check=True,
                )

                if bb == 7:
                    # theta = sigmoid(z) = 0.5 + 0.5*tanh(z/2); Tanh+Sin share
                    # an act table set.
                    tg = sm.tile([NC, 8 * 60], FP16, tag="tg", name="tg")
                    nc.scalar.activation(tg, par_g, AF.Tanh, scale=0.5)
                    co_dst = mkap(co_all, g * 8, [[1, 8], [NB, 60]])
                    si_dst = mkap(si_all, g * 8, [[1, 8], [NB, 60]])
                    tg_v = mkap(tg, 0, [[60, 8], [1, 60]])
                    nc.scalar.activation(
                        co_dst, tg_v, AF.Sin,
                        bias=float(0.25 + np.pi / 2), scale=0.25,
                    )
                    nc.scalar.activation(
                        si_dst, tg_v, AF.Sin, bias=0.25, scale=0.25
                    )

            idn_s = cload("idn", idn, [128, 128])
            kbig_s = cload("kbig", kbig, [128, 18 * 128], FP16)
            owb_s = cload("owb", owb, [19, D])
            lng_s = cload("lng", lng, [BPC, D])
            lnb_s = cload("lnb", lnb, [BPC, D])
            cw1_s = cload("cw1", cw1, [128, 2 * D])
            cb1_s = cload("cb1", cb1, [1, D])
            cw2_s = cload("cw2", cw2, [128, 4])
            cb2_s = cload("cb2", cb2, [1, 2])

            # ================= batched quantum stage 1 =================
            emit_ansatz(nc.vector, nc.scalar, nc.gpsimd, st_all, B_all,
                        co_all, si_all, epool, 2, sparse=True)

            # ================= LCU -> z rows =================
            # zx = cr^T st_re + (-ci)^T st_im ; zy = ci^T st_re + cr^T st_im
            for h in range(2):
                zx_ps = ps_m.tile([1, 512], F32, tag="m")
                nc.tensor.matmul(zx_ps, cf3_s[:, 0:1],
                                 st_all[:, h * 512 : (h + 1) * 512],
                                 start=True, stop=False)
                nc.tensor.matmul(zx_ps, cf3_s[:, 2:3],
                                 st_all[:, HALF + h * 512 : HALF + (h + 1) * 512],
                                 start=False, stop=True)
                nc.vector.tensor_scalar_mul(zrow[:, h * 512 : (h + 1) * 512], zx_ps, 1.0)
            for h in range(2):
                zy_ps = ps_m.tile([1, 512], F32, tag="m")
                nc.tensor.matmul(zy_ps, cf3_s[:, 1:2],
                                 st_all[:, h * 512 : (h + 1) * 512],
                                 start=True, stop=False)
                nc.tensor.matmul(zy_ps, cf3_s[:, 0:1],
                                 st_all[:, HALF + h * 512 : HALF + (h + 1) * 512],
                                 start=False, stop=True)
                nc.vector.tensor_scalar_mul(
                    zrow[:, HALF + h * 512 : HALF + (h + 1) * 512], zy_ps, 1.0)

            # zrow [1, (ri,b,amp)] -> Zb [16 part=b, 128=(ri,amp)] via 2 DMAs
            for ri in (0, 1):
                src = bass.AP(
                    tensor=zrow.tensor, offset=zrow.offset + ri * HALF,
                    ap=[list(zrow.ap[0])] + [[DIM, NB], [1, DIM]],
                )
                nc.sync.dma_start(out=Zb[:, ri * DIM : (ri + 1) * DIM], in_=src)
            # Zt [128, 16] = Zb^T
            zt_ps = ps_m.tile([128, BPC], F32, tag="m")
            nc.tensor.transpose(zt_ps, Zb, idn_s[0:BPC, 0:BPC])
            nc.vector.tensor_copy(Zt, zt_ps)

            # n = z.z per b; qfeat_o = (z^T K_o z) / n
            nsum = sm.tile([BPC, 1], F32, tag="nsum")
            scr0 = sm.tile([BPC, 128], F32, tag="scr0", name="scr0")
            nc.vector.scalar_tensor_tensor(
                scr0, Zb, 1.0, Zb, ALU.mult, ALU.mult, accum_out=nsum
            )
            scr = [sm.tile([BPC, 128], F32, tag=f"scrk{i}", name=f"scrk{i}")
                   for i in range(4)]
            for blk in range(5):
                n0 = blk * 4
                nobs = min(4, 18 - n0)
                y_ps = ps_t.tile([BPC, 512], F32, tag="t")
                nc.tensor.matmul(
                    y_ps[:, 0 : nobs * 128],
                    Zt[:, :],
                    kbig_s[:, n0 * 128 : (n0 + nobs) * 128],
                    start=True, stop=True,
                )
                for o in range(nobs):
                    nc.vector.scalar_tensor_tensor(
                        scr[o % 4],
                        y_ps[:, o * 128 : (o + 1) * 128],
                        1.0,
                        Zb,
                        ALU.mult, ALU.mult,
                        accum_out=qfeat[:, n0 + o : n0 + o + 1],
                    )
            rn = sm.tile([BPC, 1], F32, tag="rn")
            nc.vector.reciprocal(rn, nsum)
            nc.vector.tensor_scalar_mul(qfeat[:, 0:18], qfeat[:, 0:18], rn)

            # ================= tail =================
            qfT_ps = ps_m.tile([19, BPC], F32, tag="m")
            nc.tensor.transpose(qfT_ps, qfeat, idn_s[0:BPC, 0:BPC])
            qfT = sm.tile([19, BPC], F32, tag="qfTs")
            nc.vector.tensor_copy(qfT, qfT_ps)
            o1 = ps_t.tile([BPC, D], F32, tag="t")
            nc.tensor.matmul(o1, qfT, owb_s, start=True, stop=True)

            stats = sm.tile([BPC, 6], F32, tag="stats")
            nc.vector.bn_stats(stats, o1)
            mv = sm.tile([BPC, 2], F32, tag="mv")
            nc.vector.bn_aggr(mv, stats)
            sdv = sm.tile([BPC, 1], F32, tag="sdv")
            nc.scalar.activation(sdv, mv[:, 1:2], AF.Sqrt, bias=1e-5)
            rstd = sm.tile([BPC, 1], F32, tag="rstd")
            nc.vector.reciprocal(rstd, sdv)
            ln1 = sm.tile([BPC, D], F32, tag="ln1")
            nc.vector.tensor_scalar(
                ln1, o1, mv[:, 0:1], rstd, ALU.subtract, ALU.mult
            )
            ln2 = sm.tile([BPC, D], F32, tag="ln2")
            nc.vector.tensor_tensor(ln2, ln1, lng_s, ALU.mult)
            nc.vector.tensor_tensor(ln2, ln2, lnb_s, ALU.add)

            # cls layer 1
            lnT = [None, None]
            for h in range(2):
                lnT_ps = ps_m.tile([128, BPC], F32, tag="m")
                nc.tensor.transpose(
                    lnT_ps, ln2[:, h * 128 : (h + 1) * 128], idn_s[0:BPC, 0:BPC]
                )
                lnT[h] = sm.tile([128, BPC], F32, tag=f"lnT{h}", name=f"lnT{h}")
                nc.vector.tensor_copy(lnT[h], lnT_ps)
            h2p = ps_t.tile([BPC, D], F32, tag="t")
            nc.tensor.matmul(h2p, lnT[0], cw1_s[:, 0:D], start=True, stop=False)
            nc.tensor.matmul(
                h2p, lnT[1], cw1_s[:, D : 2 * D], start=False, stop=False
            )
            nc.tensor.matmul(
                h2p, ones[:, 0:BPC], cb1_s, start=False, stop=True
            )
            h2 = sm.tile([BPC, D], F32, tag="h2")
            nc.scalar.activation(h2, h2p, AF.Relu)

            # cls layer 2
            h2T = [None, None]
            for h in range(2):
                h2T_ps = ps_m.tile([128, BPC], F32, tag="m")
                nc.tensor.transpose(
                    h2T_ps, h2[:, h * 128 : (h + 1) * 128], idn_s[0:BPC, 0:BPC]
                )
                h2T[h] = sm.tile([128, BPC], F32, tag=f"h2T{h}", name=f"h2T{h}")
                nc.vector.tensor_copy(h2T[h], h2T_ps)
            lg = ps_t.tile([BPC, 2], F32, tag="t")
            nc.tensor.matmul(lg, h2T[0], cw2_s[:, 0:2], start=True, stop=False)
            nc.tensor.matmul(lg, h2T[1], cw2_s[:, 2:4], start=False, stop=False)
            nc.tensor.matmul(lg, ones[:, 0:BPC], cb2_s, start=False, stop=True)
            lgs = sm.tile([BPC, 2], F32, tag="lgs")
            nc.vector.tensor_copy(lgs, lg)
            nc.sync.dma_start(out=out[:, :], in_=lgs)

    if split_waits:
        _split_multi_waits(nc)
    return nc


_NC_CACHE = {}


def _get_program():
    if "nc" not in _NC_CACHE:
        _NC_CACHE["nc"] = build_program()
    return _NC_CACHE["nc"]


# ------------------------------------------------- host-side qff K matrices
def _np_rx(t):
    c, s = np.cos(t / 2), np.sin(t / 2)
    return np.array([[c, -1j * s], [-1j * s, c]])


def _np_ry(t):
    c, s = np.cos(t / 2), np.sin(t / 2)
    return np.array([[c, -s], [s, c]], complex)


def _np_rz(t):
    e = np.exp(-0.5j * t)
    return np.array([[e, 0], [0, np.conj(e)]])


def _np_apply_1q(state, gate, wire):
    # state [B, 64]; wire 0 = MSB
    dl, dr = 2 ** wire, 2 ** (NQ - 1 - wire)
    s = state.reshape(-1, dl, 2, dr)
    s = np.einsum('kj,bljr->blkr', gate, s)
    return s.reshape(-1, DIM)


def _np_apply_crx(state, theta, control, target):
    s = state.reshape((-1,) + (2,) * NQ)
    s = np.moveaxis(s, (control + 1, target + 1), (1, 2))
    g = _np_rx(theta)
    s1 = np.einsum('kj,bj...->bk...', g, s[:, 1])
    s = np.concatenate([s[:, :1], s1[:, None]], axis=1)
    s = np.moveaxis(s, (1, 2), (control + 1, target + 1))
    return s.reshape(-1, DIM)


def _np_ansatz(state, params, n_layers):
    idx = 0
    for _ in range(n_layers):
        for i in range(NQ):
            state = _np_apply_1q(state, _np_rx(params[idx]), i)
            state = _np_apply_1q(state, _np_ry(params[idx + 1]), i)
            state = _np_apply_1q(state, _np_rz(params[idx + 2]), i)
            idx += 3
        for i in range(NQ):
            state = _np_apply_crx(state, params[idx], i, (i + 1) % NQ)
            idx += 1
        for i in range(NQ - 1, -1, -1):
            state = _np_apply_crx(state, params[idx], i, (i - 1) % NQ)
            idx += 1
    return state


def _qff_k_matrices(qff_params):
    """Kbig [128, 18*128]: real quadratic forms for U^H O_o U, o = X0..X5,Y0..Y5,Z0..Z5."""
    U = _np_ansatz(np.eye(DIM, dtype=complex), np.asarray(qff_params, np.float64), 1)
    U = U.T  # rows of _np_ansatz output are evolved basis states -> columns of U
    PX = np.array([[0, 1], [1, 0]], complex)
    PY = np.array([[0, -1j], [1j, 0]])
    PZ = np.array([[1, 0], [0, -1]], complex)
    I2 = np.eye(2)
    blocks = []
    for P in (PX, PY, PZ):
        for i in range(NQ):
            O = np.array([[1.0 + 0j]])
            for w in range(NQ):
                O = np.kron(O, P if w == i else I2)
            M = U.conj().T @ O @ U
            A, Bm = M.real, M.imag
            K = np.block([[A, -Bm], [Bm, A]])
            blocks.append(K)
    return np.concatenate(blocks, axis=1).astype(np.float32)


def host_prep(inputs):
    """Host-side parameter folding -> per-core input maps."""
    f32 = np.float32
    x = np.asarray(inputs["x"], f32)
    emb_w = np.asarray(inputs["emb_w"], np.float64)
    emb_b = np.asarray(inputs["emb_b"], np.float64)
    att_w1 = np.asarray(inputs["att_w1"], np.float64)
    att_b1 = np.asarray(inputs["att_b1"], np.float64)

    bf16 = ml_dtypes.bfloat16
    wfold = (emb_w @ att_w1).astype(f32)  # [64, 128]
    bfold = (emb_b @ att_w1 + att_b1).astype(f32)  # [128]
    wfb = np.concatenate([wfold, bfold[None, :]], 0).astype(bf16)  # [65, 128]

    ewb = np.concatenate(
        [emb_w.astype(f32), emb_b.astype(f32)[None, :]], 0
    ).astype(bf16)  # [65, 256]

    pw = np.asarray(inputs["proj_w"], f32)  # [256, 60]
    pjw = np.concatenate([pw[0:128, :], pw[128:256, :]], 1).astype(bf16)  # [128, 120]

    cr = np.asarray(inputs["mix_re"], np.float64)
    ci = np.asarray(inputs["mix_im"], np.float64)
    den = np.sqrt(cr * cr + ci * ci).sum() + 1e-8
    cf3 = np.stack([cr / den, ci / den, -ci / den], 1).astype(np.float16)  # [128, 3]

    kbig = _qff_k_matrices(inputs["qff_params"]).astype(np.float16)  # [128, 2304]

    owb = np.concatenate(
        [np.asarray(inputs["out_w"], f32), np.asarray(inputs["out_b"], f32)[None, :]],
        0,
    )  # [19, 256]
    lng = np.broadcast_to(np.asarray(inputs["ln_g"], f32), (BPC, D)).copy()
    lnb = np.broadcast_to(np.asarray(inputs["ln_b"], f32), (BPC, D)).copy()
    w1 = np.asarray(inputs["cls_w1"], f32)
    cw1 = np.concatenate([w1[0:128, :], w1[128:256, :]], 1)  # [128, 512]
    cb1 = np.asarray(inputs["cls_b1"], f32)[None, :]
    w2 = np.asarray(inputs["cls_w2"], f32)
    cw2 = np.concatenate([w2[0:128, :], w2[128:256, :]], 1)  # [128, 4]
    cb2 = np.asarray(inputs["cls_b2"], f32)[None, :]
    idn = np.eye(128, dtype=f32)
    idn16 = np.eye(128, dtype=np.float16)
    pjb = np.asarray(inputs["proj_b"], f32)[None, :]

    shared = dict(
        wfb=wfb, aw2=np.asarray(inputs["att_w2"], f32).astype(bf16), ewb=ewb,
        pjw=pjw, pjb=pjb.astype(bf16), cf3=cf3, kbig=kbig, owb=owb, lng=lng,
        lnb=lnb, cw1=cw1, cb1=cb1, cw2=cw2, cb2=cb2, idn=idn, idn16=idn16,
    )

    in_maps = []
    for c in range(N_CORES):
        xc = x[c * BPC : (c + 1) * BPC]  # [16, 64, 2048]
        # xperm[b, nc, c*16+k] = x[b, c, nc*16+k]
        xp_c = np.ascontiguousarray(
            xc.reshape(BPC, C_IN, NC, CH).transpose(0, 2, 1, 3).reshape(
                BPC, NC, CH * C_IN
            )
        )
        m = dict(shared)
        m["xs"] = np.ascontiguousarray(xc).astype(bf16)
        m["xp"] = xp_c.astype(np.float16)
        in_maps.append(m)
    return in_maps


def kernel(**inputs):
    nc = _get_program()
    in_maps = host_prep(inputs)
    res = run_bass_kernel_spmd(nc, in_maps, core_ids=list(range(N_CORES)))
    outs = [res.results[c]["out"] for c in range(N_CORES)]
    return np.concatenate(outs, 0).astype(np.float32)


if __name__ == "__main__":
    nc = build_program()
    print("program built ok")
